# revision 27
# baseline (speedup 1.0000x reference)
"""3-layer GAT on Trainium2, 8 NeuronCores.

Strategy (graph/data parallel, dst-sharded).  The kernel is bound by
dma_gather's per-row DMA descriptors (HBM random-read latency), so the
design minimizes gathered rows and keeps all 4 SWDGE queue rings loaded:

  - Destination nodes are dealt round-robin (by degree rank) across 8
    cores; each core owns LOCAL=6272 node slots (6250 real + 22 pad).
  - Per layer: each core computes h = x @ W for its nodes (plus attention
    scalars s = h.a_src, d = h.a_dst via host-precomputed W@a columns),
    stages a bf16 row [h | 1 | 0 | s_f32] per node (512B rows for
    fo=128, 256B otherwise), and publishes it via FOUR partial
    AllGathers (chunked by node-tile range) that overlap the previous
    layer's edge phase; next-layer node tiles are emitted inline in the
    edge loop so the engines actually interleave them.
  - Edge layout is slot-major with lane = destination: chunk = one slot
    for the 128 destinations of a tile, so d is a per-partition bias,
    the segment softmax is a per-partition row op, and aggregation is a
    per-chunk diag(q) matmul accumulated in PSUM (denominator = the
    constant-1 row column).  All per-chunk diag matrices of a batch are
    built in ONE DVE tensor_tensor using stride-0 broadcast APs.
  - Self-loop edges are never gathered: their rows are core-local, added
    per tile as one extra matmul (diag(q_self) x [h | 1] re-read densely
    from the slice).
  - dma_gather indices are int16, so the 50176-row table is addressed
    through THREE overlapping 32768-row windows (bases 0, BHI/2, BHI =
    plain in_ap offsets); each edge is assigned to a window stream by a
    per-tile balance optimizer, cutting slot padding to ~25% over the
    single-stream floor.  Gathers are split in half and striped over the
    4 SWDGE queues: SDMA engines round-robin rings at packet granularity,
    which overlaps the HBM read latency of different rings.
  - Softmax max-subtraction is skipped (max |e| ~ 9 here, exp is safe in
    fp32); padding slots gather a dummy pad row whose s = -1e30 (q = 0)
    and one = 1.0 (keeps the patched row's self-loop denominator at 1).
"""

import os
import sys

for _p in ("/opt/trn_rl_repo", "/opt/pypackages"):
    if os.path.isdir(_p) and _p not in sys.path:
        sys.path.insert(0, _p)

import ml_dtypes
import numpy as np

import concourse.bacc as bacc
import concourse.bass as bass
import concourse.mybir as mybir
import concourse.tile as tile
from concourse.bass_utils import run_bass_kernel_spmd
from concourse.masks import make_identity

F32 = mybir.dt.float32
BF16 = mybir.dt.bfloat16
I16 = mybir.dt.int16
AF = mybir.ActivationFunctionType
ALU = mybir.AluOpType
AXL = mybir.AxisListType

P = 128
S_NEG = -1e30


def default_cfg():
    return dict(
        N=50000,
        C=8,
        DIMS=(128, 128, 64, 40),
        LO_WIN=32768,
        CAP=24,  # target gathered chunks per batch (single tiles may exceed)
        PF=(4, 4, 4),  # per-layer gather-prefetch depth in batches
        G=16,  # gather piece size in chunks (uniform SWDGE work quanta)
        NEG_SLOPE=0.2,
        ROWB=(256, 128, 128),  # bf16 slots per table row, per layer
    )


def _derived(cfg):
    N, C = cfg["N"], cfg["C"]
    assert N % C == 0
    tiles = (N // C + P - 1) // P
    local = tiles * P
    rtot = C * local
    bhi = max(0, rtot - cfg["LO_WIN"])
    return tiles, local, rtot, bhi


# AllGather chunking: the table is built by 4 partial AllGathers over
# node-tile ranges so each chunk can launch as soon as its node tiles are
# staged (hiding the collective under the previous layer's edge phase).
# Each chunk's output block is [core0-rows | core1-rows | ...]; the
# pad-bearing chunk (last tile range) is placed SECOND in table order so
# some core's pad row lands in the lo/hi index overlap [BHI, LO_WIN) and
# can serve as the dummy gather target.
AG_BOUNDS = (0, 13, 26, 38, 49)  # node-tile range boundaries
AG_TORD = (0, 3, 1, 2)  # range index -> position in table order


def _ag_blocks(cfg):
    """Returns [(lo_local, hi_local, table_base)] indexed by range id."""
    C = cfg["C"]
    ranges = [
        (AG_BOUNDS[i] * P, AG_BOUNDS[i + 1] * P) for i in range(4)
    ]
    base = 0
    tb = {}
    for ri in AG_TORD:
        lo, hi = ranges[ri]
        tb[ri] = base
        base += (hi - lo) * C
    return [(lo, hi, tb[ri]) for ri, (lo, hi) in enumerate(ranges)]


def _row_of(core, local, cfg):
    """Table row of (core, local) under the chunked-AllGather layout."""
    core, local = np.broadcast_arrays(np.asarray(core), np.asarray(local))
    row = np.empty(local.shape, np.int64)
    for lo, hi, tbase in _ag_blocks(cfg):
        m = (local >= lo) & (local < hi)
        row[m] = tbase + core[m] * (hi - lo) + (local[m] - lo)
    return row


def preprocess(edge_index, cfg):
    """Host-side graph scheduling.  Returns a dict of per-core arrays and
    the (core-uniform) tile schedule.

    Edges are split across THREE overlapping 32768-row index windows
    (bases 0, BHI/2, BHI) -- just three views of the same table, so the
    int16 gather indices can address all RTOT rows while the per-tile
    slot maxima are balanced close to the single-stream floor.
    """
    N, C = cfg["N"], cfg["C"]
    TILES, LOCAL, RTOT, BHI = _derived(cfg)
    W = cfg["LO_WIN"]
    assert BHI % 2 == 0
    BASES = (0, BHI // 2, BHI)
    NS = 3

    # Self-loop edges (the reference appends one per node) are NOT put in
    # the gather streams: their source rows are core-local, so the kernel
    # adds the q_self * [h | 1] contribution with one extra per-tile
    # matmul instead of gathering 50k rows.
    src = np.asarray(edge_index[0], dtype=np.int64)
    dst = np.asarray(edge_index[1], dtype=np.int64)
    E = src.shape[0]
    deg = np.bincount(dst, minlength=N)

    def deal(order):
        core_of = np.empty(N, np.int64)
        local_of = np.empty(N, np.int64)
        r = np.arange(N)
        core_of[order] = r % C
        local_of[order] = r // C
        return core_of, local_of

    # pass 1: rows from degree sort; pass 2 re-sorts with the fixed-w1 count
    order = np.argsort(deg, kind="stable")
    core_of, local_of = deal(order)
    rows = _row_of(core_of, local_of, cfg)
    a = np.bincount(dst[rows[src] < BASES[1]], minlength=N)
    order = np.lexsort((a, deg))
    core_of, local_of = deal(order)
    rows = _row_of(core_of, local_of, cfg)

    srow = rows[src]
    # zones: 0=[0,B2) w1 | 1=[B2,B3) w12 | 2=[B3,W) w123 | 3=[W,B2+W) w23
    #        4=[B2+W,RTOT) w3
    zone = np.digitize(srow, [BASES[1], BASES[2], W, BASES[1] + W])
    zc = np.zeros((5, N), np.int64)
    for z in range(5):
        zc[z] = np.bincount(dst[zone == z], minlength=N)
    assert np.all(zc.sum(0) == deg)
    f1, f12, f123, f23, f3 = zc

    tile_of = local_of // P

    # per-tile stream depths (uniform across cores): scan (U1, U3) budget
    # pairs, greedily draining the less-flexible zones first; U2 takes the
    # remainder.
    US = np.zeros((NS, TILES), np.int64)
    X12 = np.zeros(N, np.int64)
    X13 = np.zeros(N, np.int64)
    X23 = np.zeros(N, np.int64)
    X33 = np.zeros(N, np.int64)
    for t in range(TILES):
        m = tile_of == t
        g1, g12, g123, g23, g3 = (x[m] for x in zc)
        best = None
        lo1 = int(g1.max())
        hi1 = int((g1 + g12 + g123).max())
        lo3 = int(g3.max())
        hi3 = int((g3 + g23 + g123).max())
        for U1 in range(lo1, hi1 + 1):
            x12 = np.clip(U1 - g1, 0, g12)
            x13 = np.clip(U1 - g1 - x12, 0, g123)
            rem = g123 - x13
            for U3 in range(lo3, hi3 + 1):
                x23 = np.clip(U3 - g3, 0, g23)
                x33 = np.clip(U3 - g3 - x23, 0, rem)
                c2 = (g12 - x12) + (rem - x33) + (g23 - x23)
                tot = U1 + U3 + int(c2.max())
                if best is None or tot < best[0]:
                    best = (tot, U1, U3, x12, x13, x23, x33)
        _, U1, U3, x12, x13, x23, x33 = best
        US[0, t] = U1
        US[2, t] = U3
        US[1, t] = best[0] - U1 - U3
        X12[m] = x12
        X13[m] = x13
        X23[m] = x23
        X33[m] = x33

    # per-edge stream assignment: within (dst, zone) order, the first
    # X.. edges go to the outer streams, the rest to stream 2.
    eorder = np.lexsort((zone, dst))
    sd = dst[eorder]
    sz = zone[eorder]
    sval = srow[eorder]
    # position within the (dst, zone) run
    zstarts = np.zeros((5, N), np.int64)
    runb = np.cumsum(zc, axis=1)  # not aligned; compute per-edge instead
    key = sd * 5 + sz
    kcounts = np.bincount(key, minlength=5 * N)
    kstart = np.zeros(5 * N + 1, np.int64)
    np.cumsum(kcounts, out=kstart[1:])
    posz = np.arange(E, dtype=np.int64) - kstart[key]
    stream = np.empty(E, np.int8)
    stream[sz == 0] = 0
    stream[sz == 4] = 2
    m1 = sz == 1
    stream[m1] = np.where(posz[m1] < X12[sd[m1]], 0, 1)
    m2 = sz == 2
    p2 = posz[m2]
    s2d = sd[m2]
    stream[m2] = np.where(
        p2 < X13[s2d], 0, np.where(p2 < X13[s2d] + X33[s2d], 2, 1))
    m3 = sz == 3
    stream[m3] = np.where(posz[m3] < X23[sd[m3]], 2, 1)

    # slot within (dst, stream)
    key2 = sd * NS + stream
    k2counts = np.bincount(key2, minlength=NS * N)
    k2start = np.zeros(NS * N + 1, np.int64)
    np.cumsum(k2counts, out=k2start[1:])
    sorder = np.argsort(key2, kind="stable")
    slot = np.empty(E, np.int64)
    slot[sorder] = np.arange(E, dtype=np.int64) - k2start[key2[sorder]]

    cums = np.zeros((NS, TILES + 1), np.int64)
    np.cumsum(US, axis=1, out=cums[:, 1:])
    CH = [int(cums[s, -1]) for s in range(NS)]

    # dummy row: a pad row inside every window, i.e. [BHI, W)
    n_real = N // C
    dummy = None
    if n_real < LOCAL:
        for c in range(C):
            r0 = int(_row_of(c, n_real, cfg))
            if BHI <= r0 < min(W, RTOT):
                dummy = r0
                break
    assert dummy is not None, "no pad row available for the dummy entry"

    lane = local_of[sd] % P
    tl = tile_of[sd]
    cr = core_of[sd]

    streams = []
    for s in range(NS):
        arr = np.full((C, CH[s] * P), dummy - BASES[s], np.int64)
        ms = stream == s
        pos = (cums[s, tl[ms]] + slot[ms]) * P + lane[ms]
        arr[cr[ms], pos] = sval[ms] - BASES[s]
        assert arr.min() >= 0 and (arr.size == 0 or arr.max() < 32768)
        assert (sval[ms] >= BASES[s]).all() and \
            (sval[ms] < BASES[s] + W).all()
        streams.append(arr)

    def wrap(sarr):
        # stream position i -> [i % 16, i // 16]; the 16-partition block is
        # replicated to all 8 GPSIMD core groups (128 partitions).
        L = sarr.shape[1]
        if L == 0:
            return np.zeros((C, 128, 0), np.int16)
        w = np.ascontiguousarray(
            sarr.reshape(C, L // 16, 16).transpose(0, 2, 1)
        ).astype(np.int16)
        return np.tile(w, (1, 8, 1))

    # batches of tiles with bounded chunk totals (a single tile may exceed
    # the target on its own)
    Utot = US.sum(axis=0)
    batches = []
    t0 = 0
    while t0 < TILES:
        t1 = t0
        tot = 0
        while t1 < TILES and (t1 == t0 or tot + Utot[t1] <= cfg["CAP"]):
            tot += Utot[t1]
            t1 += 1
        batches.append((t0, t1))
        t0 = t1

    return dict(
        core_of=core_of,
        local_of=local_of,
        US=US,
        cums=cums,
        CH=CH,
        BASES=BASES,
        batches=batches,
        idx=[wrap(s) for s in streams],
        dummy=dummy,
        E_pad=sum(CH) * P,
    )


def build_program(cfg, sched):
    """Emit the (core-uniform) Bass program."""
    N, C = cfg["N"], cfg["C"]
    DIMS = cfg["DIMS"]
    TILES, LOCAL, RTOT, BHI = _derived(cfg)
    US, cums, CH = sched["US"], sched["cums"], sched["CH"]
    BASES = sched["BASES"]
    NS = len(BASES)
    batches = sched["batches"]
    CAP = cfg["CAP"]
    ROWB = cfg["ROWB"]
    NEG = cfg["NEG_SLOPE"]
    F_LAST = DIMS[3]
    n_real = N // C
    ag_blocks = _ag_blocks(cfg)

    nc = bacc.Bacc(
        "TRN2", target_bir_lowering=False, debug=False, num_devices=C,
        num_swdge_queues=4, dynamic_dma_scratch_size=24576,
    )

    # ---- I/O ----
    # x / W / intermediate node features all bf16: matches the bf16 table
    # precision downstream, halves the SBUF/DMA footprint of the node phase
    x_t_in = nc.dram_tensor("x_t", [P, LOCAL], BF16, kind="ExternalInput")
    w_in = [
        nc.dram_tensor(f"wfull{l}", [DIMS[l], DIMS[l + 1] + 2], BF16,
                       kind="ExternalInput")
        for l in range(3)
    ]
    bb_in = [
        nc.dram_tensor(f"bb{l}", [P, DIMS[l + 1]], F32, kind="ExternalInput")
        for l in range(3)
    ]
    idx_in = [
        nc.dram_tensor(f"idx{s}", [128, max(CH[s] * 8, 8)], I16,
                       kind="ExternalInput")
        for s in range(NS)
    ]
    dums_in = nc.dram_tensor("dums", [3, 256], BF16, kind="ExternalInput")
    out_d = nc.dram_tensor("out_local", [LOCAL, F_LAST], F32,
                           kind="ExternalOutput")

    with tile.TileContext(nc) as tc:
        with tc.tile_pool(name="consts", bufs=1) as cp, \
             tc.tile_pool(name="dram", bufs=1, space="DRAM") as dp, \
             tc.tile_pool(name="work", bufs=3) as wp, \
             tc.tile_pool(name="small", bufs=4) as rp, \
             tc.tile_pool(name="psA", bufs=2, space="PSUM") as psA, \
             tc.tile_pool(name="psB", bufs=2, space="PSUM") as psB, \
             tc.tile_pool(name="psC", bufs=4, space="PSUM") as psC:

            # ---- constants ----
            ident32 = cp.tile([P, P], F32, tag="ident32")
            make_identity(nc, ident32)
            identbf = cp.tile([P, P], BF16, tag="identbf")
            nc.vector.tensor_copy(identbf[:, :], ident32[:, :])

            w_sb = []
            bb_sb = []
            for l in range(3):
                wt = cp.tile([DIMS[l], DIMS[l + 1] + 2], BF16, tag=f"w{l}",
                             name=f"w_sb{l}")
                nc.sync.dma_start(wt[:, :], w_in[l][:, :])
                w_sb.append(wt)
                bt = cp.tile([P, DIMS[l + 1]], F32, tag=f"bb{l}",
                             name=f"bb_sb{l}")
                nc.sync.dma_start(bt[:, :], bb_in[l][:, :])
                bb_sb.append(bt)

            idx_sb = []
            for s in range(NS):
                it = cp.tile([128, max(CH[s] * 8, 8)], I16, tag=f"idx{s}")
                nc.sync.dma_start(it[:, :], idx_in[s][:, :])
                idx_sb.append(it)
            dums_sb = cp.tile([3, 256], BF16, tag="dums_sb")
            nc.sync.dma_start(dums_sb[:, :], dums_in[:, :])

            # s/d attention scalars of the core's own nodes, interleaved
            # [s_t0, d_t0, s_t1, d_t1, ...] (one copy per node tile).
            sd_all = [
                cp.tile([P, 2 * TILES], F32, tag=f"sd{l}", name=f"sd{l}")
                for l in range(3)
            ]
            nxt = [
                cp.tile([P, TILES * DIMS[l + 1]], BF16, tag=f"nxt{l}",
                        name=f"nxt{l}")
                for l in range(2)
            ]

            slices = [
                dp.tile([LOCAL, ROWB[l]], BF16, tag=f"slice{l}",
                        name=f"slice{l}")
                for l in range(3)
            ]
            # NOTE: addr_space="Shared" crashes NRT under the axon/PJRT
            # runtime (NRT_EXEC_UNIT_UNRECOVERABLE); Local-space output
            # works (bass warns it is slower).
            tables = [
                dp.tile([RTOT, ROWB[l]], BF16, tag=f"table{l}",
                        name=f"table{l}")
                for l in range(3)
            ]

            rg = [list(range(C))]

            def emit_ag(l, ri):
                lo, hi, tbase = ag_blocks[ri]
                nc.gpsimd.collective_compute(
                    "AllGather",
                    ALU.bypass,
                    replica_groups=rg,
                    ins=[slices[l][lo:hi, :].opt()],
                    outs=[tables[l][tbase:tbase + (hi - lo) * C, :].opt()],
                )

            ag_pending = []

            def emit_node_tile(l, t, defer_ag=False):
                """Stage tile t of layer l's node phase (h = x @ W etc.),
                then fire any AllGather chunk that just became ready."""
                fi, fo = DIMS[l], DIMS[l + 1]
                if l == 0:
                    xT = wp.tile([P, P], BF16, tag="xT0")
                    nc.sync.dma_start(
                        xT[:, :], x_t_in[:, t * P:(t + 1) * P])
                    xT_ap = xT[:fi, :]
                else:
                    xv = nxt[l - 1][:, t * fi:(t + 1) * fi]
                    xT_ps = psA.tile([P, P], BF16, tag="xT_ps")
                    nc.tensor.transpose(xT_ps[:fi, :], xv, identbf[:, :])
                    xT = wp.tile([P, P], BF16, tag="xTn")
                    nc.scalar.copy(xT[:fi, :], xT_ps[:fi, :])
                    xT_ap = xT[:fi, :]

                h_ps = psB.tile([P, 130], F32, tag="h_ps")
                nc.tensor.matmul(h_ps[:, 0:fo + 2], lhsT=xT_ap,
                                 rhs=w_sb[l][:, :], start=True, stop=True)
                nc.vector.tensor_copy(
                    sd_all[l][:, 2 * t:2 * t + 2], h_ps[:, fo:fo + 2])

                stg = wp.tile([P, ROWB[l]], BF16, tag=f"stg{l}")
                nc.scalar.copy(stg[:, 0:fo], h_ps[:, 0:fo])
                nc.vector.memset(stg[:, fo:fo + 1], 1.0)
                nc.vector.memset(stg[:, fo + 1:fo + 2], 0.0)
                nc.vector.tensor_copy(
                    stg[:, fo + 2:fo + 4].bitcast(F32),
                    h_ps[:, fo:fo + 1])
                nc.vector.memset(stg[:, fo + 4:ROWB[l]], 0.0)
                nc.sync.dma_start(
                    slices[l][t * P:(t + 1) * P, :], stg[:, :])

                # AllGather chunks in readiness order; the last one also
                # needs the dummy-row patch (pad row n_real, s = -1e30 so
                # padding slots gather q = 0; every core patches its own
                # copy, only one core's is ever addressed).
                if t + 1 == TILES:
                    nc.sync.dma_start(
                        slices[l][n_real:n_real + 1, :],
                        dums_sb[l:l + 1, 0:ROWB[l]])
                for ri in range(4):
                    if t + 1 == AG_BOUNDS[ri + 1]:
                        if defer_ag:
                            ag_pending.append((l, ri))
                        else:
                            emit_ag(l, ri)

            # ---------- layer-0 node phase ----------
            for t in range(TILES):
                emit_node_tile(0, t)

            # ---------- layers ----------
            NB = len(batches)
            MAXB = int(max(US.sum(axis=0)[t0:t1].sum()
                           for (t0, t1) in batches))
            G = cfg["G"]
            for l in range(3):
                fi, fo = DIMS[l], DIMS[l + 1]
                PF = cfg["PF"][l]
                # per-stream SBUF ring capacity: the chunk span of PF+2
                # consecutive batches (+G margin) so a gather piece never
                # waits on its ring slots' previous reader
                RS = []
                slack = PF + 1 if l == 0 else PF + 5
                for s in range(NS):
                    w = 0
                    for bi in range(NB):
                        b2 = min(bi + slack, NB - 1)
                        w = max(w, int(cums[s, batches[b2][1]]
                                       - cums[s, batches[bi][0]]))
                    RS.append(min(CH[s], w + G))
                with tc.tile_pool(name=f"mbuf{l}", bufs=1) as mp, \
                     tc.tile_pool(name=f"ebuf{l}", bufs=3) as ep, \
                     tc.tile_pool(name=f"hsl{l}", bufs=2 * PF + 6) as hsp, \
                     tc.tile_pool(name=f"qsl{l}", bufs=1) as qsp:
                    rings = [
                        mp.tile([P, RS[s], ROWB[l]], BF16, tag=f"ring{s}",
                                name=f"ring{l}_{s}")
                        for s in range(NS)
                    ]

                    # batched self-loop attention: q_self per (lane, tile)
                    sdv = sd_all[l][:, :]
                    s_ap = bass.AP(sdv.tensor, sdv.offset,
                                   [list(sdv.ap[0]), [2, TILES]])
                    d_ap = bass.AP(sdv.tensor, sdv.offset + 1,
                                   [list(sdv.ap[0]), [2, TILES]])
                    es = qsp.tile([P, TILES], F32, tag="es")
                    nc.vector.tensor_tensor(
                        out=es[:, :], in0=s_ap, in1=d_ap, op=ALU.add)
                    lre_s = qsp.tile([P, TILES], F32, tag="lres")
                    nc.scalar.activation(lre_s[:, :], es[:, :], AF.Relu)
                    nc.vector.scalar_tensor_tensor(
                        out=lre_s[:, :], in0=lre_s[:, :],
                        scalar=(1.0 - NEG) / NEG, in1=es[:, :],
                        op0=ALU.mult, op1=ALU.add)
                    qs = qsp.tile([P, TILES], F32, tag="qs")
                    nc.scalar.activation(qs[:, :], lre_s[:, :], AF.Exp,
                                         scale=NEG)

                    qctr = 0
                    hs_all = {}
                    pipe = {}

                    def issue_piece(s, c0, c1, l=l):
                        """One uniform gather piece into stream s's ring.
                        Splits at the ring boundary (rare)."""
                        nonlocal qctr
                        a = c0
                        while a < c1:
                            sl = a % RS[s]
                            n = min(c1 - a, RS[s] - sl)
                            nc.gpsimd.dma_gather(
                                out_ap=rings[s][:, sl:sl + n, :],
                                in_ap=tables[l][BASES[s]:
                                                BASES[s] + 32768, :],
                                idxs_ap=idx_sb[s][:, a * 8:(a + n) * 8],
                                num_idxs=P * n,
                                num_idxs_reg=P * n,
                                elem_size=ROWB[l],
                                single_packet=False,
                                queue_num=qctr % 4,
                            )
                            qctr += 1
                            a += n

                    def stage_a(bi, l=l, fo=fo):
                        """Attention coefficients for batch bi:
                        extraction -> lrelu -> exp -> diag build."""
                        t0, t1 = batches[bi]
                        ns_b = [int(cums[s, t1] - cums[s, t0])
                                for s in range(NS)]
                        boff = np.concatenate([[0], np.cumsum(ns_b)])
                        nch = int(boff[-1])
                        sdv = sd_all[l][:, :]

                        # per-(tile, substream) biased s extraction into
                        # batch-ordered zz/lre (split at ring wrap)
                        zzb = ep.tile([P, MAXB], F32, tag="zzb")
                        lreb = ep.tile([P, MAXB], F32, tag="lreb")
                        qb = ep.tile([P, MAXB], BF16, tag="qb")
                        sqall = ep.tile([P, MAXB * P], BF16, tag="sqall")
                        for t in range(t0, t1):
                            dcol = bass.AP(
                                sdv.tensor, sdv.offset + 2 * t + 1,
                                [list(sdv.ap[0]), [1, 1]])
                            for s in range(NS):
                                us = int(US[s, t])
                                if us == 0:
                                    continue
                                g0 = int(cums[s, t])
                                cb = int(boff[s]) + g0 - int(cums[s, t0])
                                a = 0
                                while a < us:
                                    sl = (g0 + a) % RS[s]
                                    n = min(us - a, RS[s] - sl)
                                    sview = rings[s][:, sl:sl + n,
                                                     fo + 2:fo + 4
                                                     ].bitcast(F32)
                                    nc.scalar.activation(
                                        zzb[:, cb + a:cb + a + n], sview,
                                        AF.Identity, bias=dcol)
                                    nc.scalar.activation(
                                        lreb[:, cb + a:cb + a + n], sview,
                                        AF.Relu, bias=dcol)
                                    a += n

                        # lrelu(z) = a*(z + r*relu(z)); fold a into Exp
                        nc.vector.scalar_tensor_tensor(
                            out=lreb[:, 0:nch], in0=lreb[:, 0:nch],
                            scalar=(1.0 - NEG) / NEG, in1=zzb[:, 0:nch],
                            op0=ALU.mult, op1=ALU.add)
                        nc.scalar.activation(
                            qb[:, 0:nch], lreb[:, 0:nch], AF.Exp, scale=NEG)

                        # batched diag build: sqall[p, c*P+j] =
                        #   identbf[p, j] * qb[p, c]
                        vi2 = identbf[:, :]
                        vq2 = qb[:, :]
                        vo2 = sqall[:, :]
                        nc.vector.tensor_tensor(
                            out=bass.AP(vo2.tensor, vo2.offset,
                                        [list(vo2.ap[0]), [P, nch], [1, P]]),
                            in0=bass.AP(vi2.tensor, vi2.offset,
                                        [list(vi2.ap[0]), [0, nch], [1, P]]),
                            in1=bass.AP(vq2.tensor, vq2.offset,
                                        [list(vq2.ap[0]), [1, nch], [0, P]]),
                            op=ALU.mult)
                        pipe[bi] = dict(boff=boff, sqall=sqall, accs={})

                    def stage_b(bi, l=l, fo=fo):
                        """PSUM aggregation matmul chains for batch bi."""
                        t0, t1 = batches[bi]
                        st = pipe[bi]
                        boff, sqall = st["boff"], st["sqall"]
                        for t in range(t0, t1):
                            hs = hs_all.pop(t)
                            ents = []
                            for s in range(NS):
                                g0 = int(cums[s, t])
                                cb = int(boff[s]) + g0 - int(cums[s, t0])
                                for k in range(int(US[s, t])):
                                    ents.append((s, g0 + k, cb + k))
                            U = len(ents)

                            acc = psC.tile([P, 130], F32, tag="acc")
                            av = acc[:, 0:fo + 1]
                            nc.tensor.matmul(
                                av, lhsT=sqs[:, t * P:(t + 1) * P],
                                rhs=hs[:, 0:fo + 1],
                                start=True, stop=(U == 0))
                            for u, (s, g, cb) in enumerate(ents):
                                nc.tensor.matmul(
                                    av,
                                    lhsT=sqall[:, cb * P:(cb + 1) * P],
                                    rhs=rings[s][:, g % RS[s], 0:fo + 1],
                                    start=False, stop=(u == U - 1))
                            st["accs"][t] = acc

                    def stage_c(bi, l=l, fi=fi, fo=fo):
                        """Normalize + bias + SiLU/log-softmax + next-layer
                        node staging for batch bi."""
                        t0, t1 = batches[bi]
                        st = pipe.pop(bi)
                        for t in range(t0, t1):
                            acc = st["accs"][t]
                            rc = rp.tile([P, 1], F32, tag="rc")
                            nc.vector.reciprocal(rc[:, :], acc[:, fo:fo + 1])
                            o_sb = wp.tile([P, fo], F32, tag=f"o_sb{l}")
                            nc.vector.scalar_tensor_tensor(
                                out=o_sb[:, :], in0=acc[:, 0:fo],
                                scalar=rc[:, :], in1=bb_sb[l][:, :],
                                op0=ALU.mult, op1=ALU.add)

                            if l < 2:
                                # SiLU via the exp table: x / (1 + exp(-x))
                                ex = wp.tile([P, fo], F32, tag=f"silu{l}")
                                nc.scalar.activation(
                                    ex[:, :], o_sb[:, :], AF.Exp, scale=-1.0)
                                nc.vector.tensor_scalar(
                                    out=ex[:, :], in0=ex[:, :], scalar1=1.0,
                                    scalar2=None, op0=ALU.add)
                                nc.vector.reciprocal(ex[:, :], ex[:, :])
                                nc.vector.tensor_tensor(
                                    out=nxt[l][:, t * fo:(t + 1) * fo],
                                    in0=o_sb[:, :], in1=ex[:, :],
                                    op=ALU.mult)
                                # interleave the next layer's node phase so
                                # its AllGather chunks launch early
                                emit_node_tile(l + 1, t,
                                               defer_ag=True)
                            else:
                                mneg = rp.tile([P, 1], F32, tag="mneg")
                                nc.vector.tensor_reduce(
                                    mneg[:, :], o_sb[:, :], axis=AXL.X,
                                    op=ALU.max, negate=True)
                                ex2 = wp.tile([P, fo], F32, tag="ls_e")
                                se = rp.tile([P, 1], F32, tag="se")
                                nc.scalar.activation(
                                    ex2[:, :], o_sb[:, :], AF.Exp,
                                    bias=mneg[:, :], accum_out=se[:, :])
                                lse = rp.tile([P, 1], F32, tag="lse")
                                nc.scalar.activation(
                                    lse[:, :], se[:, :], AF.Ln)
                                fin = wp.tile([P, fo], F32, tag="fin")
                                nc.vector.tensor_scalar(
                                    out=fin[:, :], in0=o_sb[:, :],
                                    scalar1=mneg[:, :], scalar2=lse[:, :],
                                    op0=ALU.add, op1=ALU.subtract)
                                nc.sync.dma_start(
                                    out_d[t * P:(t + 1) * P, :], fin[:, :])

                    # software pipeline: uniform G-chunk gather pieces issue
                    # PF batches ahead of the compute, strictly round-robin
                    # over the 4 SWDGE queues.  Uniformity matters: the Pool
                    # queue is in-order and each queue serializes on its
                    # previous gather's drain, so unequal pieces head-of-line
                    # block the rotation (per-queue drain is ~8 ns/row; 4
                    # continuously-loaded queues sustain ~2.2 ns/row).
                    # AllGather chunks flush AFTER the gather issue so an
                    # unmet AG wait never stalls gathers queued behind it.
                    pieces = []
                    for s in range(NS):
                        for c0 in range(0, CH[s], G):
                            bi = int(np.searchsorted(
                                cums[s, [b[0] for b in batches]], c0,
                                side="right")) - 1
                            pieces.append((bi, s, c0, min(c0 + G, CH[s])))
                    pieces.sort(key=lambda p: (p[0], p[1], p[2]))
                    pi = 0
                    # stage skew: by the time an op is EMITTED, its deps are
                    # already satisfied, so no in-order engine queue ever
                    # parks: gathers(b) ... coeffs(b-PF+1), matmuls(b-PF),
                    # postprocess(b-PF-1).
                    for step in range(NB + PF + 1):
                        while pi < len(pieces) and pieces[pi][0] <= step:
                            issue_piece(pieces[pi][1], pieces[pi][2],
                                        pieces[pi][3])
                            pi += 1
                        if step < NB:
                            t0, t1 = batches[step]
                            for t in range(t0, t1):
                                hs_t = hsp.tile([P, ROWB[l]], BF16,
                                                tag="hs")
                                nc.sync.dma_start(
                                    hs_t[:, :],
                                    slices[l][t * P:(t + 1) * P, :])
                                hs_all[t] = hs_t
                        while ag_pending:
                            emit_ag(*ag_pending.pop(0))
                        if PF - 1 <= step < NB + PF - 1:
                            stage_a(step - PF + 1)
                        if PF <= step < NB + PF:
                            stage_b(step - PF)
                        if step >= PF + 1:
                            stage_c(step - PF - 1)
                    assert pi == len(pieces)
                    assert not pipe and not hs_all

                    # flush any AllGather chunk still pending at layer end
                    # (the next layer's gathers wait on it; leaving it
                    # behind them on the serial Pool engine would deadlock)
                    while ag_pending:
                        emit_ag(*ag_pending.pop(0))

    nc.compile()
    return nc


def make_inputs(x, weights, cfg, sched):
    """Build the per-core in_maps."""
    N, C = cfg["N"], cfg["C"]
    TILES, LOCAL, RTOT, BHI = _derived(cfg)
    DIMS = cfg["DIMS"]
    core_of, local_of = sched["core_of"], sched["local_of"]

    x = np.asarray(x, np.float32)
    common = {}
    for l in range(3):
        W = np.asarray(weights[f"W{l}"], np.float64)
        a_s = np.asarray(weights[f"a_src{l}"], np.float64)
        a_d = np.asarray(weights[f"a_dst{l}"], np.float64)
        wfull = np.concatenate(
            [W, (W @ a_s)[:, None], (W @ a_d)[:, None]], axis=1)
        common[f"wfull{l}"] = np.ascontiguousarray(
            wfull.astype(np.float32).astype(ml_dtypes.bfloat16))
        b = np.asarray(weights[f"b{l}"], np.float32)
        common[f"bb{l}"] = np.ascontiguousarray(
            np.broadcast_to(b, (P, DIMS[l + 1])), dtype=np.float32)
    dums = np.zeros((3, 256), np.uint16)
    sneg = np.array([S_NEG], np.float32).view(np.uint16)
    for l in range(3):
        fo = DIMS[l + 1]
        # one = 1.0 keeps the patched pad row's SELF-loop denominator at 1
        # (gathered dummy slots still contribute 0: their q = exp(-inf));
        # without it lane n_real%P of the last tile divides by zero and the
        # NaN poisons real lanes via 0*NaN in the diag matmuls.
        dums[l, fo] = 0x3F80  # bf16 1.0
        dums[l, fo + 2:fo + 4] = sneg
    common["dums"] = dums.view(ml_dtypes.bfloat16).copy()

    in_maps = []
    for c in range(C):
        m = dict(common)
        nodes = np.where(core_of == c)[0]
        xt = np.zeros((P, LOCAL), np.float32)
        xt[:, local_of[nodes]] = x[nodes].T
        m["x_t"] = xt.astype(ml_dtypes.bfloat16)
        for s in range(len(sched["BASES"])):
            m[f"idx{s}"] = np.ascontiguousarray(
                sched["idx"][s][c] if sched["CH"][s] else
                np.zeros((128, 8), np.int16))
        in_maps.append(m)
    return in_maps


LAST_EXEC_NS = None
LAST_RESULTS = None


def run(inputs, cfg=None, trace=False):
    global LAST_EXEC_NS, LAST_RESULTS
    cfg = cfg or default_cfg()
    N, C = cfg["N"], cfg["C"]
    TILES, LOCAL, RTOT, BHI = _derived(cfg)

    sched = preprocess(np.asarray(inputs["edge_index"]), cfg)
    nc = build_program(cfg, sched)
    in_maps = make_inputs(inputs["x"], inputs, cfg, sched)

    res = run_bass_kernel_spmd(
        nc, in_maps, core_ids=list(range(C)), trace=trace,
        stitch_traces=trace,
    )
    LAST_EXEC_NS = res.exec_time_ns
    LAST_RESULTS = res

    F_LAST = cfg["DIMS"][3]
    out = np.empty((N, F_LAST), np.float32)
    core_of, local_of = sched["core_of"], sched["local_of"]
    for c in range(C):
        nodes = np.where(core_of == c)[0]
        out[nodes] = res.results[c]["out_local"][local_of[nodes]]
    return out


def kernel(**inputs):
    return run(inputs, trace=bool(int(os.environ.get("GAT_TRACE", "0"))))



# revision 34
# speedup vs baseline: 1.0133x; 1.0133x over previous
"""3-layer GAT on Trainium2, 8 NeuronCores.

Strategy (graph/data parallel, dst-sharded).  The kernel is bound by
dma_gather's per-row DMA descriptors (HBM random-read latency), so the
design minimizes gathered rows and keeps all 4 SWDGE queue rings loaded:

  - Destination nodes are dealt round-robin (by degree rank) across 8
    cores; each core owns LOCAL=6272 node slots (6250 real + 22 pad).
  - Per layer: each core computes h = x @ W for its nodes (plus attention
    scalars s = h.a_src, d = h.a_dst via host-precomputed W@a columns),
    stages a bf16 row [h | 1 | 0 | s_f32] per node (512B rows for
    fo=128, 256B otherwise), and publishes it via FOUR partial
    AllGathers (chunked by node-tile range) that overlap the previous
    layer's edge phase; next-layer node tiles are emitted inline in the
    edge loop so the engines actually interleave them.
  - Edge layout is slot-major with lane = destination: chunk = one slot
    for the 128 destinations of a tile, so d is a per-partition bias,
    the segment softmax is a per-partition row op, and aggregation is a
    per-chunk diag(q) matmul accumulated in PSUM (denominator = the
    constant-1 row column).  All per-chunk diag matrices of a batch are
    built in ONE DVE tensor_tensor using stride-0 broadcast APs.
  - Self-loop edges are never gathered: their rows are core-local, added
    per tile as one extra matmul (diag(q_self) x [h | 1] re-read densely
    from the slice).
  - dma_gather indices are int16, so the 50176-row table is addressed
    through THREE overlapping 32768-row windows (bases 0, BHI/2, BHI =
    plain in_ap offsets); each edge is assigned to a window stream by a
    per-tile balance optimizer, cutting slot padding to ~25% over the
    single-stream floor.  Gathers are split in half and striped over the
    4 SWDGE queues: SDMA engines round-robin rings at packet granularity,
    which overlaps the HBM read latency of different rings.
  - Softmax max-subtraction is skipped (max |e| ~ 9 here, exp is safe in
    fp32); padding slots gather a dummy pad row whose s = -1e30 (q = 0)
    and one = 1.0 (keeps the patched row's self-loop denominator at 1).
"""

import os
import sys

for _p in ("/opt/trn_rl_repo", "/opt/pypackages"):
    if os.path.isdir(_p) and _p not in sys.path:
        sys.path.insert(0, _p)

import ml_dtypes
import numpy as np

import concourse.bacc as bacc
import concourse.bass as bass
import concourse.mybir as mybir
import concourse.tile as tile
from concourse.bass_utils import run_bass_kernel_spmd
from concourse.masks import make_identity

F32 = mybir.dt.float32
BF16 = mybir.dt.bfloat16
I16 = mybir.dt.int16
AF = mybir.ActivationFunctionType
ALU = mybir.AluOpType
AXL = mybir.AxisListType

P = 128
S_NEG = -1e30


def default_cfg():
    return dict(
        N=50000,
        C=8,
        DIMS=(128, 128, 64, 40),
        LO_WIN=32768,
        CAP=24,  # target gathered chunks per batch (single tiles may exceed)
        PF=(4, 4, 4),  # per-layer gather-prefetch depth in batches
        G=16,  # gather piece size in chunks (uniform SWDGE work quanta)
        NEG_SLOPE=0.2,
        ROWB=(256, 128, 128),  # bf16 slots per table row, per layer
    )


def _derived(cfg):
    N, C = cfg["N"], cfg["C"]
    assert N % C == 0
    tiles = (N // C + P - 1) // P
    local = tiles * P
    rtot = C * local
    bhi = max(0, rtot - cfg["LO_WIN"])
    return tiles, local, rtot, bhi


# AllGather chunking: the table is built by 4 partial AllGathers over
# node-tile ranges so each chunk can launch as soon as its node tiles are
# staged (hiding the collective under the previous layer's edge phase).
# Each chunk's output block is [core0-rows | core1-rows | ...]; the
# pad-bearing chunk (last tile range) is placed SECOND in table order so
# some core's pad row lands in the lo/hi index overlap [BHI, LO_WIN) and
# can serve as the dummy gather target.
AG_BOUNDS = (0, 13, 26, 38, 49)  # node-tile range boundaries
AG_TORD = (0, 3, 1, 2)  # range index -> position in table order


def _ag_blocks(cfg):
    """Returns [(lo_local, hi_local, table_base)] indexed by range id."""
    C = cfg["C"]
    ranges = [
        (AG_BOUNDS[i] * P, AG_BOUNDS[i + 1] * P) for i in range(4)
    ]
    base = 0
    tb = {}
    for ri in AG_TORD:
        lo, hi = ranges[ri]
        tb[ri] = base
        base += (hi - lo) * C
    return [(lo, hi, tb[ri]) for ri, (lo, hi) in enumerate(ranges)]


def _row_of(core, local, cfg):
    """Table row of (core, local) under the chunked-AllGather layout."""
    core, local = np.broadcast_arrays(np.asarray(core), np.asarray(local))
    row = np.empty(local.shape, np.int64)
    for lo, hi, tbase in _ag_blocks(cfg):
        m = (local >= lo) & (local < hi)
        row[m] = tbase + core[m] * (hi - lo) + (local[m] - lo)
    return row


def preprocess(edge_index, cfg):
    """Host-side graph scheduling.  Returns a dict of per-core arrays and
    the (core-uniform) tile schedule.

    Edges are split across THREE overlapping 32768-row index windows
    (bases 0, BHI/2, BHI) -- just three views of the same table, so the
    int16 gather indices can address all RTOT rows while the per-tile
    slot maxima are balanced close to the single-stream floor.
    """
    N, C = cfg["N"], cfg["C"]
    TILES, LOCAL, RTOT, BHI = _derived(cfg)
    W = cfg["LO_WIN"]
    assert BHI % 2 == 0
    BASES = (0, BHI // 2, BHI)
    NS = 3

    # Self-loop edges (the reference appends one per node) are NOT put in
    # the gather streams: their source rows are core-local, so the kernel
    # adds the q_self * [h | 1] contribution with one extra per-tile
    # matmul instead of gathering 50k rows.
    src = np.asarray(edge_index[0], dtype=np.int64)
    dst = np.asarray(edge_index[1], dtype=np.int64)
    E = src.shape[0]
    deg = np.bincount(dst, minlength=N)

    def deal(order):
        core_of = np.empty(N, np.int64)
        local_of = np.empty(N, np.int64)
        r = np.arange(N)
        core_of[order] = r % C
        local_of[order] = r // C
        return core_of, local_of

    # pass 1: rows from degree sort; pass 2 re-sorts with the fixed-w1 count
    order = np.argsort(deg, kind="stable")
    core_of, local_of = deal(order)
    rows = _row_of(core_of, local_of, cfg)
    a = np.bincount(dst[rows[src] < BASES[1]], minlength=N)
    order = np.lexsort((a, deg))
    core_of, local_of = deal(order)
    rows = _row_of(core_of, local_of, cfg)

    srow = rows[src]
    # zones: 0=[0,B2) w1 | 1=[B2,B3) w12 | 2=[B3,W) w123 | 3=[W,B2+W) w23
    #        4=[B2+W,RTOT) w3
    zone = np.digitize(srow, [BASES[1], BASES[2], W, BASES[1] + W])
    zc = np.zeros((5, N), np.int64)
    for z in range(5):
        zc[z] = np.bincount(dst[zone == z], minlength=N)
    assert np.all(zc.sum(0) == deg)
    f1, f12, f123, f23, f3 = zc

    tile_of = local_of // P

    # per-tile stream depths (uniform across cores): scan (U1, U3) budget
    # pairs, greedily draining the less-flexible zones first; U2 takes the
    # remainder.
    US = np.zeros((NS, TILES), np.int64)
    X12 = np.zeros(N, np.int64)
    X13 = np.zeros(N, np.int64)
    X23 = np.zeros(N, np.int64)
    X33 = np.zeros(N, np.int64)
    for t in range(TILES):
        m = tile_of == t
        g1, g12, g123, g23, g3 = (x[m] for x in zc)
        best = None
        lo1 = int(g1.max())
        hi1 = int((g1 + g12 + g123).max())
        lo3 = int(g3.max())
        hi3 = int((g3 + g23 + g123).max())
        for U1 in range(lo1, hi1 + 1):
            x12 = np.clip(U1 - g1, 0, g12)
            x13 = np.clip(U1 - g1 - x12, 0, g123)
            rem = g123 - x13
            for U3 in range(lo3, hi3 + 1):
                x23 = np.clip(U3 - g3, 0, g23)
                x33 = np.clip(U3 - g3 - x23, 0, rem)
                c2 = (g12 - x12) + (rem - x33) + (g23 - x23)
                tot = U1 + U3 + int(c2.max())
                if best is None or tot < best[0]:
                    best = (tot, U1, U3, x12, x13, x23, x33)
        _, U1, U3, x12, x13, x23, x33 = best
        US[0, t] = U1
        US[2, t] = U3
        US[1, t] = best[0] - U1 - U3
        X12[m] = x12
        X13[m] = x13
        X23[m] = x23
        X33[m] = x33

    # per-edge stream assignment: within (dst, zone) order, the first
    # X.. edges go to the outer streams, the rest to stream 2.
    eorder = np.lexsort((zone, dst))
    sd = dst[eorder]
    sz = zone[eorder]
    sval = srow[eorder]
    # position within the (dst, zone) run
    zstarts = np.zeros((5, N), np.int64)
    runb = np.cumsum(zc, axis=1)  # not aligned; compute per-edge instead
    key = sd * 5 + sz
    kcounts = np.bincount(key, minlength=5 * N)
    kstart = np.zeros(5 * N + 1, np.int64)
    np.cumsum(kcounts, out=kstart[1:])
    posz = np.arange(E, dtype=np.int64) - kstart[key]
    stream = np.empty(E, np.int8)
    stream[sz == 0] = 0
    stream[sz == 4] = 2
    m1 = sz == 1
    stream[m1] = np.where(posz[m1] < X12[sd[m1]], 0, 1)
    m2 = sz == 2
    p2 = posz[m2]
    s2d = sd[m2]
    stream[m2] = np.where(
        p2 < X13[s2d], 0, np.where(p2 < X13[s2d] + X33[s2d], 2, 1))
    m3 = sz == 3
    stream[m3] = np.where(posz[m3] < X23[sd[m3]], 2, 1)

    # slot within (dst, stream)
    key2 = sd * NS + stream
    k2counts = np.bincount(key2, minlength=NS * N)
    k2start = np.zeros(NS * N + 1, np.int64)
    np.cumsum(k2counts, out=k2start[1:])
    sorder = np.argsort(key2, kind="stable")
    slot = np.empty(E, np.int64)
    slot[sorder] = np.arange(E, dtype=np.int64) - k2start[key2[sorder]]

    cums = np.zeros((NS, TILES + 1), np.int64)
    np.cumsum(US, axis=1, out=cums[:, 1:])
    CH = [int(cums[s, -1]) for s in range(NS)]

    # dummy row: a pad row inside every window, i.e. [BHI, W)
    n_real = N // C
    dummy = None
    if n_real < LOCAL:
        for c in range(C):
            r0 = int(_row_of(c, n_real, cfg))
            if BHI <= r0 < min(W, RTOT):
                dummy = r0
                break
    assert dummy is not None, "no pad row available for the dummy entry"

    lane = local_of[sd] % P
    tl = tile_of[sd]
    cr = core_of[sd]

    streams = []
    for s in range(NS):
        arr = np.full((C, CH[s] * P), dummy - BASES[s], np.int64)
        ms = stream == s
        pos = (cums[s, tl[ms]] + slot[ms]) * P + lane[ms]
        arr[cr[ms], pos] = sval[ms] - BASES[s]
        assert arr.min() >= 0 and (arr.size == 0 or arr.max() < 32768)
        assert (sval[ms] >= BASES[s]).all() and \
            (sval[ms] < BASES[s] + W).all()
        streams.append(arr)

    def wrap(sarr):
        # stream position i -> [i % 16, i // 16]; the 16-partition block is
        # replicated to all 8 GPSIMD core groups (128 partitions).
        L = sarr.shape[1]
        if L == 0:
            return np.zeros((C, 128, 0), np.int16)
        w = np.ascontiguousarray(
            sarr.reshape(C, L // 16, 16).transpose(0, 2, 1)
        ).astype(np.int16)
        return np.tile(w, (1, 8, 1))

    # batches of tiles with bounded chunk totals (a single tile may exceed
    # the target on its own)
    Utot = US.sum(axis=0)
    batches = []
    t0 = 0
    while t0 < TILES:
        t1 = t0
        tot = 0
        while t1 < TILES and (t1 == t0 or tot + Utot[t1] <= cfg["CAP"]):
            tot += Utot[t1]
            t1 += 1
        batches.append((t0, t1))
        t0 = t1

    return dict(
        core_of=core_of,
        local_of=local_of,
        US=US,
        cums=cums,
        CH=CH,
        BASES=BASES,
        batches=batches,
        idx=[wrap(s) for s in streams],
        dummy=dummy,
        E_pad=sum(CH) * P,
    )


def build_program(cfg, sched):
    """Emit the (core-uniform) Bass program."""
    N, C = cfg["N"], cfg["C"]
    DIMS = cfg["DIMS"]
    TILES, LOCAL, RTOT, BHI = _derived(cfg)
    US, cums, CH = sched["US"], sched["cums"], sched["CH"]
    BASES = sched["BASES"]
    NS = len(BASES)
    batches = sched["batches"]
    CAP = cfg["CAP"]
    ROWB = cfg["ROWB"]
    NEG = cfg["NEG_SLOPE"]
    F_LAST = DIMS[3]
    n_real = N // C
    ag_blocks = _ag_blocks(cfg)

    nc = bacc.Bacc(
        "TRN2", target_bir_lowering=False, debug=False, num_devices=C,
        num_swdge_queues=4, dynamic_dma_scratch_size=24576,
    )

    # ---- I/O ----
    # x / W / intermediate node features all bf16: matches the bf16 table
    # precision downstream, halves the SBUF/DMA footprint of the node phase
    x_t_in = nc.dram_tensor("x_t", [P, LOCAL], BF16, kind="ExternalInput")
    w_in = [
        nc.dram_tensor(f"wfull{l}", [DIMS[l], DIMS[l + 1] + 2], BF16,
                       kind="ExternalInput")
        for l in range(3)
    ]
    bb_in = [
        nc.dram_tensor(f"bb{l}", [P, DIMS[l + 1]], F32, kind="ExternalInput")
        for l in range(3)
    ]
    idx_in = [
        nc.dram_tensor(f"idx{s}", [128, max(CH[s] * 8, 8)], I16,
                       kind="ExternalInput")
        for s in range(NS)
    ]
    dums_in = nc.dram_tensor("dums", [3, 256], BF16, kind="ExternalInput")
    out_d = nc.dram_tensor("out_local", [LOCAL, F_LAST], F32,
                           kind="ExternalOutput")

    with tile.TileContext(nc) as tc:
        with tc.tile_pool(name="consts", bufs=1) as cp, \
             tc.tile_pool(name="dram", bufs=1, space="DRAM") as dp, \
             tc.tile_pool(name="work", bufs=3) as wp, \
             tc.tile_pool(name="small", bufs=4) as rp, \
             tc.tile_pool(name="psA", bufs=2, space="PSUM") as psA, \
             tc.tile_pool(name="psB", bufs=2, space="PSUM") as psB, \
             tc.tile_pool(name="psC", bufs=4, space="PSUM") as psC:

            # ---- constants ----
            ident32 = cp.tile([P, P], F32, tag="ident32")
            make_identity(nc, ident32)
            identbf = cp.tile([P, P], BF16, tag="identbf")
            nc.vector.tensor_copy(identbf[:, :], ident32[:, :])

            w_sb = []
            bb_sb = []
            for l in range(3):
                wt = cp.tile([DIMS[l], DIMS[l + 1] + 2], BF16, tag=f"w{l}",
                             name=f"w_sb{l}")
                nc.sync.dma_start(wt[:, :], w_in[l][:, :])
                w_sb.append(wt)
                bt = cp.tile([P, DIMS[l + 1]], F32, tag=f"bb{l}",
                             name=f"bb_sb{l}")
                nc.sync.dma_start(bt[:, :], bb_in[l][:, :])
                bb_sb.append(bt)

            idx_sb = []
            for s in range(NS):
                it = cp.tile([128, max(CH[s] * 8, 8)], I16, tag=f"idx{s}")
                nc.sync.dma_start(it[:, :], idx_in[s][:, :])
                idx_sb.append(it)
            dums_sb = cp.tile([3, 256], BF16, tag="dums_sb")
            nc.sync.dma_start(dums_sb[:, :], dums_in[:, :])

            # s/d attention scalars of the core's own nodes, interleaved
            # [s_t0, d_t0, s_t1, d_t1, ...] (one copy per node tile).
            sd_all = [
                cp.tile([P, 2 * TILES], F32, tag=f"sd{l}", name=f"sd{l}")
                for l in range(3)
            ]
            nxt = [
                cp.tile([P, TILES * DIMS[l + 1]], BF16, tag=f"nxt{l}",
                        name=f"nxt{l}")
                for l in range(2)
            ]

            slices = [
                dp.tile([LOCAL, ROWB[l]], BF16, tag=f"slice{l}",
                        name=f"slice{l}")
                for l in range(3)
            ]
            # NOTE: addr_space="Shared" crashes NRT under the axon/PJRT
            # runtime (NRT_EXEC_UNIT_UNRECOVERABLE); Local-space output
            # works (bass warns it is slower).
            tables = [
                dp.tile([RTOT, ROWB[l]], BF16, tag=f"table{l}",
                        name=f"table{l}")
                for l in range(3)
            ]

            rg = [list(range(C))]

            def emit_ag(l, ri):
                lo, hi, tbase = ag_blocks[ri]
                nc.gpsimd.collective_compute(
                    "AllGather",
                    ALU.bypass,
                    replica_groups=rg,
                    ins=[slices[l][lo:hi, :].opt()],
                    outs=[tables[l][tbase:tbase + (hi - lo) * C, :].opt()],
                )

            ag_pending = []

            def emit_node_tile(l, t, defer_ag=False):
                """Stage tile t of layer l's node phase (h = x @ W etc.),
                then fire any AllGather chunk that just became ready."""
                fi, fo = DIMS[l], DIMS[l + 1]
                if l == 0:
                    xT = wp.tile([P, P], BF16, tag="xT0")
                    nc.sync.dma_start(
                        xT[:, :], x_t_in[:, t * P:(t + 1) * P])
                    xT_ap = xT[:fi, :]
                else:
                    xv = nxt[l - 1][:, t * fi:(t + 1) * fi]
                    xT_ps = psA.tile([P, P], BF16, tag="xT_ps")
                    nc.tensor.transpose(xT_ps[:fi, :], xv, identbf[:, :])
                    xT = wp.tile([P, P], BF16, tag="xTn")
                    nc.scalar.copy(xT[:fi, :], xT_ps[:fi, :])
                    xT_ap = xT[:fi, :]

                h_ps = psB.tile([P, 130], F32, tag="h_ps")
                nc.tensor.matmul(h_ps[:, 0:fo + 2], lhsT=xT_ap,
                                 rhs=w_sb[l][:, :], start=True, stop=True)
                nc.vector.tensor_copy(
                    sd_all[l][:, 2 * t:2 * t + 2], h_ps[:, fo:fo + 2])

                stg = wp.tile([P, ROWB[l]], BF16, tag=f"stg{l}")
                nc.scalar.copy(stg[:, 0:fo], h_ps[:, 0:fo])
                nc.vector.memset(stg[:, fo:fo + 1], 1.0)
                nc.vector.memset(stg[:, fo + 1:fo + 2], 0.0)
                nc.vector.tensor_copy(
                    stg[:, fo + 2:fo + 4].bitcast(F32),
                    h_ps[:, fo:fo + 1])
                nc.vector.memset(stg[:, fo + 4:ROWB[l]], 0.0)
                nc.sync.dma_start(
                    slices[l][t * P:(t + 1) * P, :], stg[:, :])

                # AllGather chunks in readiness order; the last one also
                # needs the dummy-row patch (pad row n_real, s = -1e30 so
                # padding slots gather q = 0; every core patches its own
                # copy, only one core's is ever addressed).
                if t + 1 == TILES:
                    nc.sync.dma_start(
                        slices[l][n_real:n_real + 1, :],
                        dums_sb[l:l + 1, 0:ROWB[l]])
                for ri in range(4):
                    if t + 1 == AG_BOUNDS[ri + 1]:
                        if defer_ag:
                            ag_pending.append((l, ri))
                        else:
                            emit_ag(l, ri)

            # ---------- layer-0 node phase ----------
            for t in range(TILES):
                emit_node_tile(0, t)

            # ---------- layers ----------
            NB = len(batches)
            MAXB = int(max(US.sum(axis=0)[t0:t1].sum()
                           for (t0, t1) in batches))
            G = cfg["G"]
            for l in range(3):
                fi, fo = DIMS[l], DIMS[l + 1]
                PF = cfg["PF"][l]
                # per-stream SBUF ring capacity: the chunk span of PF+2
                # consecutive batches (+G margin) so a gather piece never
                # waits on its ring slots' previous reader
                RS = []
                slack = PF + 2 if l == 0 else PF + 5
                for s in range(NS):
                    w = 0
                    for bi in range(NB):
                        b2 = min(bi + slack, NB - 1)
                        w = max(w, int(cums[s, batches[b2][1]]
                                       - cums[s, batches[bi][0]]))
                    RS.append(min(CH[s], w + G))
                with tc.tile_pool(name=f"mbuf{l}", bufs=1) as mp, \
                     tc.tile_pool(name=f"ebuf{l}",
                                  bufs=(2 if l == 0 else 3)) as ep, \
                     tc.tile_pool(name=f"hsl{l}", bufs=2 * PF + 6) as hsp, \
                     tc.tile_pool(name=f"qsl{l}", bufs=1) as qsp:
                    rings = [
                        mp.tile([P, RS[s], ROWB[l]], BF16, tag=f"ring{s}",
                                name=f"ring{l}_{s}")
                        for s in range(NS)
                    ]

                    # batched self-loop attention: q_self per (lane, tile)
                    sdv = sd_all[l][:, :]
                    s_ap = bass.AP(sdv.tensor, sdv.offset,
                                   [list(sdv.ap[0]), [2, TILES]])
                    d_ap = bass.AP(sdv.tensor, sdv.offset + 1,
                                   [list(sdv.ap[0]), [2, TILES]])
                    es = qsp.tile([P, TILES], F32, tag="es")
                    nc.vector.tensor_tensor(
                        out=es[:, :], in0=s_ap, in1=d_ap, op=ALU.add)
                    lre_s = qsp.tile([P, TILES], F32, tag="lres")
                    nc.scalar.activation(lre_s[:, :], es[:, :], AF.Relu)
                    nc.vector.scalar_tensor_tensor(
                        out=lre_s[:, :], in0=lre_s[:, :],
                        scalar=(1.0 - NEG) / NEG, in1=es[:, :],
                        op0=ALU.mult, op1=ALU.add)
                    qs = qsp.tile([P, TILES], F32, tag="qs")
                    nc.scalar.activation(qs[:, :], lre_s[:, :], AF.Exp,
                                         scale=NEG)

                    qctr = 0
                    hs_all = {}
                    pipe = {}

                    def issue_piece(s, c0, c1, l=l):
                        """One uniform gather piece into stream s's ring.
                        Splits at the ring boundary (rare)."""
                        nonlocal qctr
                        a = c0
                        while a < c1:
                            sl = a % RS[s]
                            n = min(c1 - a, RS[s] - sl)
                            nc.gpsimd.dma_gather(
                                out_ap=rings[s][:, sl:sl + n, :],
                                in_ap=tables[l][BASES[s]:
                                                BASES[s] + 32768, :],
                                idxs_ap=idx_sb[s][:, a * 8:(a + n) * 8],
                                num_idxs=P * n,
                                num_idxs_reg=P * n,
                                elem_size=ROWB[l],
                                single_packet=False,
                                queue_num=qctr % 4,
                            )
                            qctr += 1
                            a += n

                    def stage_a(bi, l=l, fo=fo):
                        """Attention coefficients for batch bi: extraction
                        -> lrelu -> exp -> q-scaled rows (replaces the old
                        diag-matrix build, whose stride-0 broadcast ran
                        element-serial on DVE at ~20us a batch)."""
                        t0, t1 = batches[bi]
                        ns_b = [int(cums[s, t1] - cums[s, t0])
                                for s in range(NS)]
                        boff = np.concatenate([[0], np.cumsum(ns_b)])
                        nch = int(boff[-1])
                        sdv = sd_all[l][:, :]

                        # per-(tile, substream) biased s extraction into
                        # batch-ordered zz/lre (split at ring wrap)
                        zzb = ep.tile([P, MAXB], F32, tag="zzb")
                        lreb = ep.tile([P, MAXB], F32, tag="lreb")
                        qb = ep.tile([P, MAXB], F32, tag="qb")
                        for t in range(t0, t1):
                            dcol = bass.AP(
                                sdv.tensor, sdv.offset + 2 * t + 1,
                                [list(sdv.ap[0]), [1, 1]])
                            for s in range(NS):
                                us = int(US[s, t])
                                if us == 0:
                                    continue
                                g0 = int(cums[s, t])
                                cb = int(boff[s]) + g0 - int(cums[s, t0])
                                a = 0
                                while a < us:
                                    sl = (g0 + a) % RS[s]
                                    n = min(us - a, RS[s] - sl)
                                    sview = rings[s][:, sl:sl + n,
                                                     fo + 2:fo + 4
                                                     ].bitcast(F32)
                                    nc.scalar.activation(
                                        zzb[:, cb + a:cb + a + n], sview,
                                        AF.Identity, bias=dcol)
                                    nc.scalar.activation(
                                        lreb[:, cb + a:cb + a + n], sview,
                                        AF.Relu, bias=dcol)
                                    a += n

                        # lrelu(z) = a*(z + r*relu(z)); fold a into Exp
                        nc.vector.scalar_tensor_tensor(
                            out=lreb[:, 0:nch], in0=lreb[:, 0:nch],
                            scalar=(1.0 - NEG) / NEG, in1=zzb[:, 0:nch],
                            op0=ALU.mult, op1=ALU.add)
                        nc.scalar.activation(
                            qb[:, 0:nch], lreb[:, 0:nch], AF.Exp, scale=NEG)

                        # q-scaled rows: scl[:, cb, :] = ring_row * q[cb]
                        # (per-partition scalar multiply; the aggregation
                        # matmuls then all share the constant identity lhsT)
                        nt = t1 - t0
                        scl = ep.tile([P, (MAXB + 2) * (fo + 1)], BF16,
                                      tag="scl")
                        for t in range(t0, t1):
                            for s in range(NS):
                                us = int(US[s, t])
                                if us == 0:
                                    continue
                                g0 = int(cums[s, t])
                                cb = int(boff[s]) + g0 - int(cums[s, t0])
                                a = 0
                                while a < us:
                                    sl = (g0 + a) % RS[s]
                                    n = min(us - a, RS[s] - sl)
                                    for k in range(n):
                                        c = cb + a + k
                                        nc.vector.tensor_scalar(
                                            out=scl[:, c * (fo + 1):
                                                    (c + 1) * (fo + 1)],
                                            in0=rings[s][:, sl + k,
                                                         0:fo + 1],
                                            scalar1=qb[:, c:c + 1],
                                            scalar2=None, op0=ALU.mult)
                                    a += n
                            # self-loop row, scaled by q_self of tile t
                            hs = hs_all.pop(t)
                            c = nch + (t - t0)
                            nc.vector.tensor_scalar(
                                out=scl[:, c * (fo + 1):(c + 1) * (fo + 1)],
                                in0=hs[:, 0:fo + 1],
                                scalar1=qs[:, t:t + 1],
                                scalar2=None, op0=ALU.mult)
                        pipe[bi] = dict(boff=boff, scl=scl, nch=nch,
                                        accs={})

                    def stage_b(bi, l=l, fo=fo):
                        """PSUM aggregation matmul chains for batch bi."""
                        t0, t1 = batches[bi]
                        st = pipe[bi]
                        boff, scl, nch = st["boff"], st["scl"], st["nch"]
                        for t in range(t0, t1):
                            chl = [nch + (t - t0)]  # self row first
                            for s in range(NS):
                                g0 = int(cums[s, t])
                                cb = int(boff[s]) + g0 - int(cums[s, t0])
                                chl.extend(range(cb, cb + int(US[s, t])))
                            acc = psC.tile([P, 130], F32, tag="acc")
                            av = acc[:, 0:fo + 1]
                            for u, c in enumerate(chl):
                                nc.tensor.matmul(
                                    av, lhsT=identbf[:, :],
                                    rhs=scl[:, c * (fo + 1):
                                            (c + 1) * (fo + 1)],
                                    start=(u == 0), stop=(u == len(chl) - 1))
                            st["accs"][t] = acc

                    def stage_c(bi, l=l, fi=fi, fo=fo):
                        """Normalize + bias + SiLU/log-softmax + next-layer
                        node staging for batch bi."""
                        t0, t1 = batches[bi]
                        st = pipe.pop(bi)
                        for t in range(t0, t1):
                            acc = st["accs"][t]
                            rc = rp.tile([P, 1], F32, tag="rc")
                            nc.vector.reciprocal(rc[:, :], acc[:, fo:fo + 1])
                            o_sb = wp.tile([P, fo], F32, tag=f"o_sb{l}")
                            nc.vector.scalar_tensor_tensor(
                                out=o_sb[:, :], in0=acc[:, 0:fo],
                                scalar=rc[:, :], in1=bb_sb[l][:, :],
                                op0=ALU.mult, op1=ALU.add)

                            if l < 2:
                                nc.scalar.activation(
                                    nxt[l][:, t * fo:(t + 1) * fo],
                                    o_sb[:, :], AF.Silu)
                                # interleave the next layer's node phase so
                                # its AllGather chunks launch early
                                emit_node_tile(l + 1, t,
                                               defer_ag=True)
                            else:
                                mneg = rp.tile([P, 1], F32, tag="mneg")
                                nc.vector.tensor_reduce(
                                    mneg[:, :], o_sb[:, :], axis=AXL.X,
                                    op=ALU.max, negate=True)
                                ex2 = wp.tile([P, fo], F32, tag="ls_e")
                                se = rp.tile([P, 1], F32, tag="se")
                                nc.scalar.activation(
                                    ex2[:, :], o_sb[:, :], AF.Exp,
                                    bias=mneg[:, :], accum_out=se[:, :])
                                lse = rp.tile([P, 1], F32, tag="lse")
                                nc.scalar.activation(
                                    lse[:, :], se[:, :], AF.Ln)
                                fin = wp.tile([P, fo], F32, tag="fin")
                                nc.vector.tensor_scalar(
                                    out=fin[:, :], in0=o_sb[:, :],
                                    scalar1=mneg[:, :], scalar2=lse[:, :],
                                    op0=ALU.add, op1=ALU.subtract)
                                nc.sync.dma_start(
                                    out_d[t * P:(t + 1) * P, :], fin[:, :])

                    # software pipeline: uniform G-chunk gather pieces issue
                    # PF batches ahead of the compute, strictly round-robin
                    # over the 4 SWDGE queues.  Uniformity matters: the Pool
                    # queue is in-order and each queue serializes on its
                    # previous gather's drain, so unequal pieces head-of-line
                    # block the rotation (per-queue drain is ~8 ns/row; 4
                    # continuously-loaded queues sustain ~2.2 ns/row).
                    # AllGather chunks flush AFTER the gather issue so an
                    # unmet AG wait never stalls gathers queued behind it.
                    pieces = []
                    for s in range(NS):
                        for c0 in range(0, CH[s], G):
                            bi = int(np.searchsorted(
                                cums[s, [b[0] for b in batches]], c0,
                                side="right")) - 1
                            pieces.append((bi, s, c0, min(c0 + G, CH[s])))
                    pieces.sort(key=lambda p: (p[0], p[1], p[2]))
                    pi = 0
                    ag_timed = []
                    # stage skew: by the time an op is EMITTED, its deps are
                    # already satisfied, so no in-order engine queue ever
                    # parks: gathers(b) ... coeffs(b-PF+1), matmuls(b-PF),
                    # postprocess(b-PF-1).  AllGathers flush 2 steps after
                    # their slice writes were issued so their wait is met at
                    # the Pool queue head (an unmet AG wait blocks every
                    # gather queued behind it).
                    for step in range(NB + PF + 1):
                        while pi < len(pieces) and pieces[pi][0] <= step:
                            issue_piece(pieces[pi][1], pieces[pi][2],
                                        pieces[pi][3])
                            pi += 1
                        if step < NB:
                            t0, t1 = batches[step]
                            for t in range(t0, t1):
                                hs_t = hsp.tile([P, ROWB[l]], BF16,
                                                tag="hs")
                                nc.sync.dma_start(
                                    hs_t[:, :],
                                    slices[l][t * P:(t + 1) * P, :])
                                hs_all[t] = hs_t
                        while ag_timed and step - ag_timed[0][0] >= 2:
                            _, al, ari = ag_timed.pop(0)
                            emit_ag(al, ari)
                        if PF - 1 <= step < NB + PF - 1:
                            stage_a(step - PF + 1)
                        if PF <= step < NB + PF:
                            stage_b(step - PF)
                        if step >= PF + 1:
                            stage_c(step - PF - 1)
                        while ag_pending:
                            ag_timed.append((step,) + ag_pending.pop(0))
                    while ag_timed:
                        _, al, ari = ag_timed.pop(0)
                        emit_ag(al, ari)
                    assert pi == len(pieces)
                    assert not pipe and not hs_all

                    # flush any AllGather chunk still pending at layer end
                    # (the next layer's gathers wait on it; leaving it
                    # behind them on the serial Pool engine would deadlock)
                    while ag_pending:
                        emit_ag(*ag_pending.pop(0))

    nc.compile()
    return nc


def make_inputs(x, weights, cfg, sched):
    """Build the per-core in_maps."""
    N, C = cfg["N"], cfg["C"]
    TILES, LOCAL, RTOT, BHI = _derived(cfg)
    DIMS = cfg["DIMS"]
    core_of, local_of = sched["core_of"], sched["local_of"]

    x = np.asarray(x, np.float32)
    common = {}
    for l in range(3):
        W = np.asarray(weights[f"W{l}"], np.float64)
        a_s = np.asarray(weights[f"a_src{l}"], np.float64)
        a_d = np.asarray(weights[f"a_dst{l}"], np.float64)
        wfull = np.concatenate(
            [W, (W @ a_s)[:, None], (W @ a_d)[:, None]], axis=1)
        common[f"wfull{l}"] = np.ascontiguousarray(
            wfull.astype(np.float32).astype(ml_dtypes.bfloat16))
        b = np.asarray(weights[f"b{l}"], np.float32)
        common[f"bb{l}"] = np.ascontiguousarray(
            np.broadcast_to(b, (P, DIMS[l + 1])), dtype=np.float32)
    dums = np.zeros((3, 256), np.uint16)
    sneg = np.array([S_NEG], np.float32).view(np.uint16)
    for l in range(3):
        fo = DIMS[l + 1]
        # one = 1.0 keeps the patched pad row's SELF-loop denominator at 1
        # (gathered dummy slots still contribute 0: their q = exp(-inf));
        # without it lane n_real%P of the last tile divides by zero and the
        # NaN poisons real lanes via 0*NaN in the diag matmuls.
        dums[l, fo] = 0x3F80  # bf16 1.0
        dums[l, fo + 2:fo + 4] = sneg
    common["dums"] = dums.view(ml_dtypes.bfloat16).copy()

    in_maps = []
    for c in range(C):
        m = dict(common)
        nodes = np.where(core_of == c)[0]
        xt = np.zeros((P, LOCAL), np.float32)
        xt[:, local_of[nodes]] = x[nodes].T
        m["x_t"] = xt.astype(ml_dtypes.bfloat16)
        for s in range(len(sched["BASES"])):
            m[f"idx{s}"] = np.ascontiguousarray(
                sched["idx"][s][c] if sched["CH"][s] else
                np.zeros((128, 8), np.int16))
        in_maps.append(m)
    return in_maps


LAST_EXEC_NS = None
LAST_RESULTS = None


def run(inputs, cfg=None, trace=False):
    global LAST_EXEC_NS, LAST_RESULTS
    cfg = cfg or default_cfg()
    N, C = cfg["N"], cfg["C"]
    TILES, LOCAL, RTOT, BHI = _derived(cfg)

    sched = preprocess(np.asarray(inputs["edge_index"]), cfg)
    nc = build_program(cfg, sched)
    in_maps = make_inputs(inputs["x"], inputs, cfg, sched)

    res = run_bass_kernel_spmd(
        nc, in_maps, core_ids=list(range(C)), trace=trace,
        stitch_traces=trace,
    )
    LAST_EXEC_NS = res.exec_time_ns
    LAST_RESULTS = res

    F_LAST = cfg["DIMS"][3]
    out = np.empty((N, F_LAST), np.float32)
    core_of, local_of = sched["core_of"], sched["local_of"]
    for c in range(C):
        nodes = np.where(core_of == c)[0]
        out[nodes] = res.results[c]["out_local"][local_of[nodes]]
    return out


def kernel(**inputs):
    return run(inputs, trace=bool(int(os.environ.get("GAT_TRACE", "0"))))



# revision 37
# speedup vs baseline: 1.0342x; 1.0205x over previous
"""3-layer GAT on Trainium2, 8 NeuronCores.

Strategy (graph/data parallel, dst-sharded).  The kernel is bound by
dma_gather's per-row DMA descriptors (HBM random-read latency), so the
design minimizes gathered rows and keeps all 4 SWDGE queue rings loaded:

  - Destination nodes are dealt round-robin (by degree rank) across 8
    cores; each core owns LOCAL=6272 node slots (6250 real + 22 pad).
  - Per layer: each core computes h = x @ W for its nodes (plus attention
    scalars s = h.a_src, d = h.a_dst via host-precomputed W@a columns),
    stages a bf16 row [h | 1 | 0 | s_f32] per node (512B rows for
    fo=128, 256B otherwise), and publishes it via FOUR partial
    AllGathers (chunked by node-tile range) that overlap the previous
    layer's edge phase; next-layer node tiles are emitted inline in the
    edge loop so the engines actually interleave them.
  - Edge layout is slot-major with lane = destination: chunk = one slot
    for the 128 destinations of a tile, so d is a per-partition bias,
    the segment softmax is a per-partition row op, and aggregation is a
    per-chunk diag(q) matmul accumulated in PSUM (denominator = the
    constant-1 row column).  All per-chunk diag matrices of a batch are
    built in ONE DVE tensor_tensor using stride-0 broadcast APs.
  - Self-loop edges are never gathered: their rows are core-local, added
    per tile as one extra matmul (diag(q_self) x [h | 1] re-read densely
    from the slice).
  - dma_gather indices are int16, so the 50176-row table is addressed
    through THREE overlapping 32768-row windows (bases 0, BHI/2, BHI =
    plain in_ap offsets); each edge is assigned to a window stream by a
    per-tile balance optimizer, cutting slot padding to ~25% over the
    single-stream floor.  Gathers are split in half and striped over the
    4 SWDGE queues: SDMA engines round-robin rings at packet granularity,
    which overlaps the HBM read latency of different rings.
  - Softmax max-subtraction is skipped (max |e| ~ 9 here, exp is safe in
    fp32); padding slots gather a dummy pad row whose s = -1e30 (q = 0)
    and one = 1.0 (keeps the patched row's self-loop denominator at 1).
"""

import os
import sys

for _p in ("/opt/trn_rl_repo", "/opt/pypackages"):
    if os.path.isdir(_p) and _p not in sys.path:
        sys.path.insert(0, _p)

import ml_dtypes
import numpy as np

import concourse.bacc as bacc
import concourse.bass as bass
import concourse.mybir as mybir
import concourse.tile as tile
from concourse.bass_utils import run_bass_kernel_spmd
from concourse.masks import make_identity

F32 = mybir.dt.float32
BF16 = mybir.dt.bfloat16
I16 = mybir.dt.int16
AF = mybir.ActivationFunctionType
ALU = mybir.AluOpType
AXL = mybir.AxisListType

P = 128
S_NEG = -1e30


def default_cfg():
    return dict(
        N=50000,
        C=8,
        DIMS=(128, 128, 64, 40),
        LO_WIN=32768,
        CAP=24,  # target gathered chunks per batch (single tiles may exceed)
        PF=(4, 4, 4),  # per-layer gather-prefetch depth in batches
        G=16,  # gather piece size in chunks (uniform SWDGE work quanta)
        NEG_SLOPE=0.2,
        ROWB=(256, 128, 128),  # bf16 slots per table row, per layer
    )


def _derived(cfg):
    N, C = cfg["N"], cfg["C"]
    assert N % C == 0
    tiles = (N // C + P - 1) // P
    local = tiles * P
    rtot = C * local
    bhi = max(0, rtot - cfg["LO_WIN"])
    return tiles, local, rtot, bhi


# AllGather chunking: the table is built by 4 partial AllGathers over
# node-tile ranges so each chunk can launch as soon as its node tiles are
# staged (hiding the collective under the previous layer's edge phase).
# Each chunk's output block is [core0-rows | core1-rows | ...]; the
# pad-bearing chunk (last tile range) is placed SECOND in table order so
# some core's pad row lands in the lo/hi index overlap [BHI, LO_WIN) and
# can serve as the dummy gather target.
AG_BOUNDS = (0, 13, 26, 38, 49)  # node-tile range boundaries
AG_TORD = (0, 3, 1, 2)  # range index -> position in table order


def _ag_blocks(cfg):
    """Returns [(lo_local, hi_local, table_base)] indexed by range id."""
    C = cfg["C"]
    ranges = [
        (AG_BOUNDS[i] * P, AG_BOUNDS[i + 1] * P) for i in range(4)
    ]
    base = 0
    tb = {}
    for ri in AG_TORD:
        lo, hi = ranges[ri]
        tb[ri] = base
        base += (hi - lo) * C
    return [(lo, hi, tb[ri]) for ri, (lo, hi) in enumerate(ranges)]


def _row_of(core, local, cfg):
    """Table row of (core, local) under the chunked-AllGather layout."""
    core, local = np.broadcast_arrays(np.asarray(core), np.asarray(local))
    row = np.empty(local.shape, np.int64)
    for lo, hi, tbase in _ag_blocks(cfg):
        m = (local >= lo) & (local < hi)
        row[m] = tbase + core[m] * (hi - lo) + (local[m] - lo)
    return row


def preprocess(edge_index, cfg):
    """Host-side graph scheduling.  Returns a dict of per-core arrays and
    the (core-uniform) tile schedule.

    Edges are split across THREE overlapping 32768-row index windows
    (bases 0, BHI/2, BHI) -- just three views of the same table, so the
    int16 gather indices can address all RTOT rows while the per-tile
    slot maxima are balanced close to the single-stream floor.
    """
    N, C = cfg["N"], cfg["C"]
    TILES, LOCAL, RTOT, BHI = _derived(cfg)
    W = cfg["LO_WIN"]
    assert BHI % 2 == 0
    BASES = (0, BHI // 2, BHI)
    NS = 3

    # Self-loop edges (the reference appends one per node) are NOT put in
    # the gather streams: their source rows are core-local, so the kernel
    # adds the q_self * [h | 1] contribution with one extra per-tile
    # matmul instead of gathering 50k rows.
    src = np.asarray(edge_index[0], dtype=np.int64)
    dst = np.asarray(edge_index[1], dtype=np.int64)
    E = src.shape[0]
    deg = np.bincount(dst, minlength=N)

    def deal(order):
        core_of = np.empty(N, np.int64)
        local_of = np.empty(N, np.int64)
        r = np.arange(N)
        core_of[order] = r % C
        local_of[order] = r // C
        return core_of, local_of

    # pass 1: rows from degree sort; pass 2 re-sorts with the fixed-w1 count
    order = np.argsort(deg, kind="stable")
    core_of, local_of = deal(order)
    rows = _row_of(core_of, local_of, cfg)
    a = np.bincount(dst[rows[src] < BASES[1]], minlength=N)
    order = np.lexsort((a, deg))
    core_of, local_of = deal(order)
    rows = _row_of(core_of, local_of, cfg)

    srow = rows[src]
    # zones: 0=[0,B2) w1 | 1=[B2,B3) w12 | 2=[B3,W) w123 | 3=[W,B2+W) w23
    #        4=[B2+W,RTOT) w3
    zone = np.digitize(srow, [BASES[1], BASES[2], W, BASES[1] + W])
    zc = np.zeros((5, N), np.int64)
    for z in range(5):
        zc[z] = np.bincount(dst[zone == z], minlength=N)
    assert np.all(zc.sum(0) == deg)
    f1, f12, f123, f23, f3 = zc

    tile_of = local_of // P

    # per-tile stream depths (uniform across cores): scan (U1, U3) budget
    # pairs, greedily draining the less-flexible zones first; U2 takes the
    # remainder.
    US = np.zeros((NS, TILES), np.int64)
    X12 = np.zeros(N, np.int64)
    X13 = np.zeros(N, np.int64)
    X23 = np.zeros(N, np.int64)
    X33 = np.zeros(N, np.int64)
    for t in range(TILES):
        m = tile_of == t
        g1, g12, g123, g23, g3 = (x[m] for x in zc)
        best = None
        lo1 = int(g1.max())
        hi1 = int((g1 + g12 + g123).max())
        lo3 = int(g3.max())
        hi3 = int((g3 + g23 + g123).max())
        for U1 in range(lo1, hi1 + 1):
            x12 = np.clip(U1 - g1, 0, g12)
            x13 = np.clip(U1 - g1 - x12, 0, g123)
            rem = g123 - x13
            for U3 in range(lo3, hi3 + 1):
                x23 = np.clip(U3 - g3, 0, g23)
                x33 = np.clip(U3 - g3 - x23, 0, rem)
                c2 = (g12 - x12) + (rem - x33) + (g23 - x23)
                tot = U1 + U3 + int(c2.max())
                if best is None or tot < best[0]:
                    best = (tot, U1, U3, x12, x13, x23, x33)
        _, U1, U3, x12, x13, x23, x33 = best
        US[0, t] = U1
        US[2, t] = U3
        US[1, t] = best[0] - U1 - U3
        X12[m] = x12
        X13[m] = x13
        X23[m] = x23
        X33[m] = x33

    # per-edge stream assignment: within (dst, zone) order, the first
    # X.. edges go to the outer streams, the rest to stream 2.
    eorder = np.lexsort((zone, dst))
    sd = dst[eorder]
    sz = zone[eorder]
    sval = srow[eorder]
    # position within the (dst, zone) run
    zstarts = np.zeros((5, N), np.int64)
    runb = np.cumsum(zc, axis=1)  # not aligned; compute per-edge instead
    key = sd * 5 + sz
    kcounts = np.bincount(key, minlength=5 * N)
    kstart = np.zeros(5 * N + 1, np.int64)
    np.cumsum(kcounts, out=kstart[1:])
    posz = np.arange(E, dtype=np.int64) - kstart[key]
    stream = np.empty(E, np.int8)
    stream[sz == 0] = 0
    stream[sz == 4] = 2
    m1 = sz == 1
    stream[m1] = np.where(posz[m1] < X12[sd[m1]], 0, 1)
    m2 = sz == 2
    p2 = posz[m2]
    s2d = sd[m2]
    stream[m2] = np.where(
        p2 < X13[s2d], 0, np.where(p2 < X13[s2d] + X33[s2d], 2, 1))
    m3 = sz == 3
    stream[m3] = np.where(posz[m3] < X23[sd[m3]], 2, 1)

    # slot within (dst, stream)
    key2 = sd * NS + stream
    k2counts = np.bincount(key2, minlength=NS * N)
    k2start = np.zeros(NS * N + 1, np.int64)
    np.cumsum(k2counts, out=k2start[1:])
    sorder = np.argsort(key2, kind="stable")
    slot = np.empty(E, np.int64)
    slot[sorder] = np.arange(E, dtype=np.int64) - k2start[key2[sorder]]

    cums = np.zeros((NS, TILES + 1), np.int64)
    np.cumsum(US, axis=1, out=cums[:, 1:])
    CH = [int(cums[s, -1]) for s in range(NS)]

    # dummy row: a pad row inside every window, i.e. [BHI, W)
    n_real = N // C
    dummy = None
    if n_real < LOCAL:
        for c in range(C):
            r0 = int(_row_of(c, n_real, cfg))
            if BHI <= r0 < min(W, RTOT):
                dummy = r0
                break
    assert dummy is not None, "no pad row available for the dummy entry"

    lane = local_of[sd] % P
    tl = tile_of[sd]
    cr = core_of[sd]

    streams = []
    for s in range(NS):
        arr = np.full((C, CH[s] * P), dummy - BASES[s], np.int64)
        ms = stream == s
        pos = (cums[s, tl[ms]] + slot[ms]) * P + lane[ms]
        arr[cr[ms], pos] = sval[ms] - BASES[s]
        assert arr.min() >= 0 and (arr.size == 0 or arr.max() < 32768)
        assert (sval[ms] >= BASES[s]).all() and \
            (sval[ms] < BASES[s] + W).all()
        streams.append(arr)

    def wrap(sarr):
        # stream position i -> [i % 16, i // 16]; the 16-partition block is
        # replicated to all 8 GPSIMD core groups (128 partitions).
        L = sarr.shape[1]
        if L == 0:
            return np.zeros((C, 128, 0), np.int16)
        w = np.ascontiguousarray(
            sarr.reshape(C, L // 16, 16).transpose(0, 2, 1)
        ).astype(np.int16)
        return np.tile(w, (1, 8, 1))

    # batches of tiles with bounded chunk totals (a single tile may exceed
    # the target on its own)
    Utot = US.sum(axis=0)
    batches = []
    t0 = 0
    while t0 < TILES:
        t1 = t0
        tot = 0
        while t1 < TILES and (t1 == t0 or tot + Utot[t1] <= cfg["CAP"]):
            tot += Utot[t1]
            t1 += 1
        batches.append((t0, t1))
        t0 = t1

    return dict(
        core_of=core_of,
        local_of=local_of,
        US=US,
        cums=cums,
        CH=CH,
        BASES=BASES,
        batches=batches,
        idx=[wrap(s) for s in streams],
        dummy=dummy,
        E_pad=sum(CH) * P,
    )


def build_program(cfg, sched):
    """Emit the (core-uniform) Bass program."""
    N, C = cfg["N"], cfg["C"]
    DIMS = cfg["DIMS"]
    TILES, LOCAL, RTOT, BHI = _derived(cfg)
    US, cums, CH = sched["US"], sched["cums"], sched["CH"]
    BASES = sched["BASES"]
    NS = len(BASES)
    batches = sched["batches"]
    CAP = cfg["CAP"]
    ROWB = cfg["ROWB"]
    NEG = cfg["NEG_SLOPE"]
    F_LAST = DIMS[3]
    n_real = N // C
    ag_blocks = _ag_blocks(cfg)

    nc = bacc.Bacc(
        "TRN2", target_bir_lowering=False, debug=False, num_devices=C,
        num_swdge_queues=4, dynamic_dma_scratch_size=24576,
    )

    # ---- I/O ----
    # x / W / intermediate node features all bf16: matches the bf16 table
    # precision downstream, halves the SBUF/DMA footprint of the node phase
    x_t_in = nc.dram_tensor("x_t", [P, LOCAL], BF16, kind="ExternalInput")
    w_in = [
        nc.dram_tensor(f"wfull{l}", [DIMS[l], DIMS[l + 1] + 2], BF16,
                       kind="ExternalInput")
        for l in range(3)
    ]
    bb_in = [
        nc.dram_tensor(f"bb{l}", [P, DIMS[l + 1]], F32, kind="ExternalInput")
        for l in range(3)
    ]
    idx_in = [
        nc.dram_tensor(f"idx{s}", [128, max(CH[s] * 8, 8)], I16,
                       kind="ExternalInput")
        for s in range(NS)
    ]
    dums_in = nc.dram_tensor("dums", [3, 256], BF16, kind="ExternalInput")
    out_d = nc.dram_tensor("out_local", [LOCAL, F_LAST], F32,
                           kind="ExternalOutput")

    with tile.TileContext(nc) as tc:
        with tc.tile_pool(name="consts", bufs=1) as cp, \
             tc.tile_pool(name="dram", bufs=1, space="DRAM") as dp, \
             tc.tile_pool(name="work", bufs=3) as wp, \
             tc.tile_pool(name="small", bufs=4) as rp, \
             tc.tile_pool(name="psA", bufs=2, space="PSUM") as psA, \
             tc.tile_pool(name="psB", bufs=2, space="PSUM") as psB, \
             tc.tile_pool(name="psC", bufs=4, space="PSUM") as psC:

            # ---- constants ----
            ident32 = cp.tile([P, P], F32, tag="ident32")
            make_identity(nc, ident32)
            identbf = cp.tile([P, P], BF16, tag="identbf")
            nc.vector.tensor_copy(identbf[:, :], ident32[:, :])

            w_sb = []
            bb_sb = []
            for l in range(3):
                wt = cp.tile([DIMS[l], DIMS[l + 1] + 2], BF16, tag=f"w{l}",
                             name=f"w_sb{l}")
                nc.sync.dma_start(wt[:, :], w_in[l][:, :])
                w_sb.append(wt)
                bt = cp.tile([P, DIMS[l + 1]], F32, tag=f"bb{l}",
                             name=f"bb_sb{l}")
                nc.sync.dma_start(bt[:, :], bb_in[l][:, :])
                bb_sb.append(bt)

            idx_sb = []
            for s in range(NS):
                it = cp.tile([128, max(CH[s] * 8, 8)], I16, tag=f"idx{s}")
                nc.sync.dma_start(it[:, :], idx_in[s][:, :])
                idx_sb.append(it)
            dums_sb = cp.tile([3, 256], BF16, tag="dums_sb")
            nc.sync.dma_start(dums_sb[:, :], dums_in[:, :])

            # s/d attention scalars of the core's own nodes, interleaved
            # [s_t0, d_t0, s_t1, d_t1, ...] (one copy per node tile).
            sd_all = [
                cp.tile([P, 2 * TILES], F32, tag=f"sd{l}", name=f"sd{l}")
                for l in range(3)
            ]
            nxt = [
                cp.tile([P, TILES * DIMS[l + 1]], BF16, tag=f"nxt{l}",
                        name=f"nxt{l}")
                for l in range(2)
            ]

            slices = [
                dp.tile([LOCAL, ROWB[l]], BF16, tag=f"slice{l}",
                        name=f"slice{l}")
                for l in range(3)
            ]
            # NOTE: addr_space="Shared" crashes NRT under the axon/PJRT
            # runtime (NRT_EXEC_UNIT_UNRECOVERABLE); Local-space output
            # works (bass warns it is slower).
            tables = [
                dp.tile([RTOT, ROWB[l]], BF16, tag=f"table{l}",
                        name=f"table{l}")
                for l in range(3)
            ]

            rg = [list(range(C))]

            def emit_ag(l, ri):
                lo, hi, tbase = ag_blocks[ri]
                nc.gpsimd.collective_compute(
                    "AllGather",
                    ALU.bypass,
                    replica_groups=rg,
                    ins=[slices[l][lo:hi, :].opt()],
                    outs=[tables[l][tbase:tbase + (hi - lo) * C, :].opt()],
                )

            ag_pending = []

            def emit_node_tile(l, t, defer_ag=False):
                """Stage tile t of layer l's node phase (h = x @ W etc.),
                then fire any AllGather chunk that just became ready."""
                fi, fo = DIMS[l], DIMS[l + 1]
                if l == 0:
                    xT = wp.tile([P, P], BF16, tag="xT0")
                    nc.sync.dma_start(
                        xT[:, :], x_t_in[:, t * P:(t + 1) * P])
                    xT_ap = xT[:fi, :]
                else:
                    xv = nxt[l - 1][:, t * fi:(t + 1) * fi]
                    xT_ps = psA.tile([P, P], BF16, tag="xT_ps")
                    nc.tensor.transpose(xT_ps[:fi, :], xv, identbf[:, :])
                    xT = wp.tile([P, P], BF16, tag="xTn")
                    nc.scalar.copy(xT[:fi, :], xT_ps[:fi, :])
                    xT_ap = xT[:fi, :]

                h_ps = psB.tile([P, 130], F32, tag="h_ps")
                nc.tensor.matmul(h_ps[:, 0:fo + 2], lhsT=xT_ap,
                                 rhs=w_sb[l][:, :], start=True, stop=True)
                nc.vector.tensor_copy(
                    sd_all[l][:, 2 * t:2 * t + 2], h_ps[:, fo:fo + 2])

                stg = wp.tile([P, ROWB[l]], BF16, tag=f"stg{l}")
                nc.scalar.copy(stg[:, 0:fo], h_ps[:, 0:fo])
                nc.vector.memset(stg[:, fo:fo + 1], 1.0)
                nc.vector.memset(stg[:, fo + 1:fo + 2], 0.0)
                nc.vector.tensor_copy(
                    stg[:, fo + 2:fo + 4].bitcast(F32),
                    h_ps[:, fo:fo + 1])
                nc.vector.memset(stg[:, fo + 4:ROWB[l]], 0.0)
                nc.sync.dma_start(
                    slices[l][t * P:(t + 1) * P, :], stg[:, :])

                # AllGather chunks in readiness order; the last one also
                # needs the dummy-row patch (pad row n_real, s = -1e30 so
                # padding slots gather q = 0; every core patches its own
                # copy, only one core's is ever addressed).
                if t + 1 == TILES:
                    nc.sync.dma_start(
                        slices[l][n_real:n_real + 1, :],
                        dums_sb[l:l + 1, 0:ROWB[l]])
                for ri in range(4):
                    if t + 1 == AG_BOUNDS[ri + 1]:
                        if defer_ag:
                            ag_pending.append((l, ri))
                        else:
                            emit_ag(l, ri)

            # ---------- layer-0 node phase ----------
            for t in range(TILES):
                emit_node_tile(0, t)

            # ---------- layers ----------
            NB = len(batches)
            MAXB = int(max(US.sum(axis=0)[t0:t1].sum()
                           for (t0, t1) in batches))
            G = cfg["G"]
            for l in range(3):
                fi, fo = DIMS[l], DIMS[l + 1]
                PF = cfg["PF"][l]
                # per-stream SBUF ring capacity: the chunk span of PF+2
                # consecutive batches (+G margin) so a gather piece never
                # waits on its ring slots' previous reader
                RS = []
                slack = PF + 2 if l == 0 else PF + 5
                for s in range(NS):
                    w = 0
                    for bi in range(NB):
                        b2 = min(bi + slack, NB - 1)
                        w = max(w, int(cums[s, batches[b2][1]]
                                       - cums[s, batches[bi][0]]))
                    RS.append(min(CH[s], w + G))
                with tc.tile_pool(name=f"mbuf{l}", bufs=1) as mp, \
                     tc.tile_pool(name=f"ebuf{l}",
                                  bufs=(2 if l == 0 else 3)) as ep, \
                     tc.tile_pool(name=f"hsl{l}", bufs=2 * PF + 6) as hsp, \
                     tc.tile_pool(name=f"qsl{l}", bufs=1) as qsp:
                    rings = [
                        mp.tile([P, RS[s], ROWB[l]], BF16, tag=f"ring{s}",
                                name=f"ring{l}_{s}")
                        for s in range(NS)
                    ]

                    # batched self-loop attention: q_self per (lane, tile)
                    sdv = sd_all[l][:, :]
                    s_ap = bass.AP(sdv.tensor, sdv.offset,
                                   [list(sdv.ap[0]), [2, TILES]])
                    d_ap = bass.AP(sdv.tensor, sdv.offset + 1,
                                   [list(sdv.ap[0]), [2, TILES]])
                    es = qsp.tile([P, TILES], F32, tag="es")
                    nc.vector.tensor_tensor(
                        out=es[:, :], in0=s_ap, in1=d_ap, op=ALU.add)
                    lre_s = qsp.tile([P, TILES], F32, tag="lres")
                    nc.scalar.activation(lre_s[:, :], es[:, :], AF.Relu)
                    nc.vector.scalar_tensor_tensor(
                        out=lre_s[:, :], in0=lre_s[:, :],
                        scalar=(1.0 - NEG) / NEG, in1=es[:, :],
                        op0=ALU.mult, op1=ALU.add)
                    qs = qsp.tile([P, TILES], F32, tag="qs")
                    nc.scalar.activation(qs[:, :], lre_s[:, :], AF.Exp,
                                         scale=NEG)
                    qsf = qsp.tile([P, TILES], BF16, tag="qsf")
                    nc.vector.tensor_copy(qsf[:, :], qs[:, :])

                    qctr = 0
                    hs_all = {}
                    pipe = {}

                    def issue_piece(s, c0, c1, l=l):
                        """One uniform gather piece into stream s's ring.
                        Splits at the ring boundary (rare)."""
                        nonlocal qctr
                        a = c0
                        while a < c1:
                            sl = a % RS[s]
                            n = min(c1 - a, RS[s] - sl)
                            nc.gpsimd.dma_gather(
                                out_ap=rings[s][:, sl:sl + n, :],
                                in_ap=tables[l][BASES[s]:
                                                BASES[s] + 32768, :],
                                idxs_ap=idx_sb[s][:, a * 8:(a + n) * 8],
                                num_idxs=P * n,
                                num_idxs_reg=P * n,
                                elem_size=ROWB[l],
                                single_packet=False,
                                queue_num=qctr % 4,
                            )
                            qctr += 1
                            a += n

                    def stage_a(bi, l=l, fo=fo):
                        """Attention coefficients for batch bi: extraction
                        -> lrelu -> exp -> q-scaled rows (replaces the old
                        diag-matrix build, whose stride-0 broadcast ran
                        element-serial on DVE at ~20us a batch)."""
                        t0, t1 = batches[bi]
                        ns_b = [int(cums[s, t1] - cums[s, t0])
                                for s in range(NS)]
                        boff = np.concatenate([[0], np.cumsum(ns_b)])
                        nch = int(boff[-1])
                        sdv = sd_all[l][:, :]

                        # per-(tile, substream) biased s extraction into
                        # batch-ordered zz/lre (split at ring wrap)
                        zzb = ep.tile([P, MAXB], F32, tag="zzb")
                        lreb = ep.tile([P, MAXB], F32, tag="lreb")
                        qb = ep.tile([P, MAXB], BF16, tag="qb")
                        for t in range(t0, t1):
                            dcol = bass.AP(
                                sdv.tensor, sdv.offset + 2 * t + 1,
                                [list(sdv.ap[0]), [1, 1]])
                            for s in range(NS):
                                us = int(US[s, t])
                                if us == 0:
                                    continue
                                g0 = int(cums[s, t])
                                cb = int(boff[s]) + g0 - int(cums[s, t0])
                                a = 0
                                while a < us:
                                    sl = (g0 + a) % RS[s]
                                    n = min(us - a, RS[s] - sl)
                                    sview = rings[s][:, sl:sl + n,
                                                     fo + 2:fo + 4
                                                     ].bitcast(F32)
                                    nc.scalar.activation(
                                        zzb[:, cb + a:cb + a + n], sview,
                                        AF.Identity, bias=dcol)
                                    nc.scalar.activation(
                                        lreb[:, cb + a:cb + a + n], sview,
                                        AF.Relu, bias=dcol)
                                    a += n

                        # lrelu(z) = a*(z + r*relu(z)); fold a into Exp
                        nc.vector.scalar_tensor_tensor(
                            out=lreb[:, 0:nch], in0=lreb[:, 0:nch],
                            scalar=(1.0 - NEG) / NEG, in1=zzb[:, 0:nch],
                            op0=ALU.mult, op1=ALU.add)
                        nc.scalar.activation(
                            qb[:, 0:nch], lreb[:, 0:nch], AF.Exp, scale=NEG)

                        # q-scaled rows: scl[:, c, :] = ring_row * q[c].
                        # One DVE tensor_tensor per contiguous span; q
                        # broadcasts along the feature axis via an
                        # innermost stride-0 AP (full DVE rate — unlike a
                        # middle-dim stride-0, which runs element-serial).
                        F1 = fo + 1
                        scl = ep.tile([P, (MAXB + 2) * F1], BF16,
                                      tag="scl")
                        pp = list(scl.ap[0])
                        for t in range(t0, t1):
                            for s in range(NS):
                                us = int(US[s, t])
                                if us == 0:
                                    continue
                                g0 = int(cums[s, t])
                                cb = int(boff[s]) + g0 - int(cums[s, t0])
                                a = 0
                                while a < us:
                                    sl = (g0 + a) % RS[s]
                                    n = min(us - a, RS[s] - sl)
                                    c = cb + a
                                    rg = rings[s]
                                    nc.vector.tensor_tensor(
                                        out=bass.AP(
                                            scl.tensor,
                                            scl.offset + c * F1,
                                            [pp, [F1, n], [1, F1]]),
                                        in0=bass.AP(
                                            rg.tensor,
                                            rg.offset + sl * ROWB[l],
                                            [list(rg.ap[0]),
                                             [ROWB[l], n], [1, F1]]),
                                        in1=bass.AP(
                                            qb.tensor, qb.offset + c,
                                            [list(qb.ap[0]),
                                             [1, n], [0, F1]]),
                                        op=ALU.mult)
                                    a += n
                            # self-loop row, scaled by q_self of tile t
                            hs = hs_all.pop(t)
                            c = nch + (t - t0)
                            nc.vector.tensor_tensor(
                                out=bass.AP(scl.tensor,
                                            scl.offset + c * F1,
                                            [pp, [1, 1], [1, F1]]),
                                in0=bass.AP(hs.tensor, hs.offset,
                                            [list(hs.ap[0]),
                                             [1, 1], [1, F1]]),
                                in1=bass.AP(qsf.tensor, qsf.offset + t,
                                            [list(qsf.ap[0]),
                                             [1, 1], [0, F1]]),
                                op=ALU.mult)
                        pipe[bi] = dict(boff=boff, scl=scl, nch=nch,
                                        accs={})

                    def stage_b(bi, l=l, fo=fo):
                        """PSUM aggregation matmul chains for batch bi."""
                        t0, t1 = batches[bi]
                        st = pipe[bi]
                        boff, scl, nch = st["boff"], st["scl"], st["nch"]
                        for t in range(t0, t1):
                            chl = [nch + (t - t0)]  # self row first
                            for s in range(NS):
                                g0 = int(cums[s, t])
                                cb = int(boff[s]) + g0 - int(cums[s, t0])
                                chl.extend(range(cb, cb + int(US[s, t])))
                            acc = psC.tile([P, 130], F32, tag="acc")
                            av = acc[:, 0:fo + 1]
                            for u, c in enumerate(chl):
                                nc.tensor.matmul(
                                    av, lhsT=identbf[:, :],
                                    rhs=scl[:, c * (fo + 1):
                                            (c + 1) * (fo + 1)],
                                    start=(u == 0), stop=(u == len(chl) - 1))
                            st["accs"][t] = acc

                    def stage_c(bi, l=l, fi=fi, fo=fo):
                        """Normalize + bias + SiLU/log-softmax + next-layer
                        node staging for batch bi."""
                        t0, t1 = batches[bi]
                        st = pipe.pop(bi)
                        for t in range(t0, t1):
                            acc = st["accs"][t]
                            rc = rp.tile([P, 1], F32, tag="rc")
                            nc.vector.reciprocal(rc[:, :], acc[:, fo:fo + 1])
                            o_sb = wp.tile([P, fo], F32, tag=f"o_sb{l}")
                            nc.vector.scalar_tensor_tensor(
                                out=o_sb[:, :], in0=acc[:, 0:fo],
                                scalar=rc[:, :], in1=bb_sb[l][:, :],
                                op0=ALU.mult, op1=ALU.add)

                            if l < 2:
                                nc.scalar.activation(
                                    nxt[l][:, t * fo:(t + 1) * fo],
                                    o_sb[:, :], AF.Silu)
                                # interleave the next layer's node phase so
                                # its AllGather chunks launch early
                                emit_node_tile(l + 1, t,
                                               defer_ag=True)
                            else:
                                mneg = rp.tile([P, 1], F32, tag="mneg")
                                nc.vector.tensor_reduce(
                                    mneg[:, :], o_sb[:, :], axis=AXL.X,
                                    op=ALU.max, negate=True)
                                ex2 = wp.tile([P, fo], F32, tag="ls_e")
                                se = rp.tile([P, 1], F32, tag="se")
                                nc.scalar.activation(
                                    ex2[:, :], o_sb[:, :], AF.Exp,
                                    bias=mneg[:, :], accum_out=se[:, :])
                                lse = rp.tile([P, 1], F32, tag="lse")
                                nc.scalar.activation(
                                    lse[:, :], se[:, :], AF.Ln)
                                fin = wp.tile([P, fo], F32, tag="fin")
                                nc.vector.tensor_scalar(
                                    out=fin[:, :], in0=o_sb[:, :],
                                    scalar1=mneg[:, :], scalar2=lse[:, :],
                                    op0=ALU.add, op1=ALU.subtract)
                                nc.sync.dma_start(
                                    out_d[t * P:(t + 1) * P, :], fin[:, :])

                    # software pipeline: uniform G-chunk gather pieces issue
                    # PF batches ahead of the compute, strictly round-robin
                    # over the 4 SWDGE queues.  Uniformity matters: the Pool
                    # queue is in-order and each queue serializes on its
                    # previous gather's drain, so unequal pieces head-of-line
                    # block the rotation (per-queue drain is ~8 ns/row; 4
                    # continuously-loaded queues sustain ~2.2 ns/row).
                    # AllGather chunks flush AFTER the gather issue so an
                    # unmet AG wait never stalls gathers queued behind it.
                    pieces = []
                    for s in range(NS):
                        for c0 in range(0, CH[s], G):
                            bi = int(np.searchsorted(
                                cums[s, [b[0] for b in batches]], c0,
                                side="right")) - 1
                            pieces.append((bi, s, c0, min(c0 + G, CH[s])))
                    pieces.sort(key=lambda p: (p[0], p[1], p[2]))
                    pi = 0
                    ag_timed = []
                    # stage skew: by the time an op is EMITTED, its deps are
                    # already satisfied, so no in-order engine queue ever
                    # parks: gathers(b) ... coeffs(b-PF+1), matmuls(b-PF),
                    # postprocess(b-PF-1).  AllGathers flush 2 steps after
                    # their slice writes were issued so their wait is met at
                    # the Pool queue head (an unmet AG wait blocks every
                    # gather queued behind it).
                    for step in range(NB + PF + 1):
                        while pi < len(pieces) and pieces[pi][0] <= step:
                            issue_piece(pieces[pi][1], pieces[pi][2],
                                        pieces[pi][3])
                            pi += 1
                        if step < NB:
                            t0, t1 = batches[step]
                            for t in range(t0, t1):
                                hs_t = hsp.tile([P, ROWB[l]], BF16,
                                                tag="hs")
                                nc.sync.dma_start(
                                    hs_t[:, :],
                                    slices[l][t * P:(t + 1) * P, :])
                                hs_all[t] = hs_t
                        while ag_timed and step - ag_timed[0][0] >= 2:
                            _, al, ari = ag_timed.pop(0)
                            emit_ag(al, ari)
                        if PF - 1 <= step < NB + PF - 1:
                            stage_a(step - PF + 1)
                        if PF <= step < NB + PF:
                            stage_b(step - PF)
                        if step >= PF + 1:
                            stage_c(step - PF - 1)
                        while ag_pending:
                            ag_timed.append((step,) + ag_pending.pop(0))
                    while ag_timed:
                        _, al, ari = ag_timed.pop(0)
                        emit_ag(al, ari)
                    assert pi == len(pieces)
                    assert not pipe and not hs_all

                    # flush any AllGather chunk still pending at layer end
                    # (the next layer's gathers wait on it; leaving it
                    # behind them on the serial Pool engine would deadlock)
                    while ag_pending:
                        emit_ag(*ag_pending.pop(0))

    nc.compile()
    return nc


def make_inputs(x, weights, cfg, sched):
    """Build the per-core in_maps."""
    N, C = cfg["N"], cfg["C"]
    TILES, LOCAL, RTOT, BHI = _derived(cfg)
    DIMS = cfg["DIMS"]
    core_of, local_of = sched["core_of"], sched["local_of"]

    x = np.asarray(x, np.float32)
    common = {}
    for l in range(3):
        W = np.asarray(weights[f"W{l}"], np.float64)
        a_s = np.asarray(weights[f"a_src{l}"], np.float64)
        a_d = np.asarray(weights[f"a_dst{l}"], np.float64)
        wfull = np.concatenate(
            [W, (W @ a_s)[:, None], (W @ a_d)[:, None]], axis=1)
        common[f"wfull{l}"] = np.ascontiguousarray(
            wfull.astype(np.float32).astype(ml_dtypes.bfloat16))
        b = np.asarray(weights[f"b{l}"], np.float32)
        common[f"bb{l}"] = np.ascontiguousarray(
            np.broadcast_to(b, (P, DIMS[l + 1])), dtype=np.float32)
    dums = np.zeros((3, 256), np.uint16)
    sneg = np.array([S_NEG], np.float32).view(np.uint16)
    for l in range(3):
        fo = DIMS[l + 1]
        # one = 1.0 keeps the patched pad row's SELF-loop denominator at 1
        # (gathered dummy slots still contribute 0: their q = exp(-inf));
        # without it lane n_real%P of the last tile divides by zero and the
        # NaN poisons real lanes via 0*NaN in the diag matmuls.
        dums[l, fo] = 0x3F80  # bf16 1.0
        dums[l, fo + 2:fo + 4] = sneg
    common["dums"] = dums.view(ml_dtypes.bfloat16).copy()

    in_maps = []
    for c in range(C):
        m = dict(common)
        nodes = np.where(core_of == c)[0]
        xt = np.zeros((P, LOCAL), np.float32)
        xt[:, local_of[nodes]] = x[nodes].T
        m["x_t"] = xt.astype(ml_dtypes.bfloat16)
        for s in range(len(sched["BASES"])):
            m[f"idx{s}"] = np.ascontiguousarray(
                sched["idx"][s][c] if sched["CH"][s] else
                np.zeros((128, 8), np.int16))
        in_maps.append(m)
    return in_maps


LAST_EXEC_NS = None
LAST_RESULTS = None


def run(inputs, cfg=None, trace=False):
    global LAST_EXEC_NS, LAST_RESULTS
    cfg = cfg or default_cfg()
    N, C = cfg["N"], cfg["C"]
    TILES, LOCAL, RTOT, BHI = _derived(cfg)

    sched = preprocess(np.asarray(inputs["edge_index"]), cfg)
    nc = build_program(cfg, sched)
    in_maps = make_inputs(inputs["x"], inputs, cfg, sched)

    res = run_bass_kernel_spmd(
        nc, in_maps, core_ids=list(range(C)), trace=trace,
        stitch_traces=trace,
    )
    LAST_EXEC_NS = res.exec_time_ns
    LAST_RESULTS = res

    F_LAST = cfg["DIMS"][3]
    out = np.empty((N, F_LAST), np.float32)
    core_of, local_of = sched["core_of"], sched["local_of"]
    for c in range(C):
        nodes = np.where(core_of == c)[0]
        out[nodes] = res.results[c]["out_local"][local_of[nodes]]
    return out


def kernel(**inputs):
    return run(inputs, trace=bool(int(os.environ.get("GAT_TRACE", "0"))))

